# revision 1
# baseline (speedup 1.0000x reference)
"""LightGCN 2-layer propagation on 8 TRN2 NeuronCores.

Layer 0 (1.6M edges, x[100000,128] -> h0[50000,128]): dst-sharded. Each core
owns 49 dst tiles of 128 rows (6272 dsts). Edge features are gathered from a
bf16 copy of x via GPSIMD dma_gather (x split into 4 chunks of 25000 rows so
indices fit int16), scaled by edge weight and segment-summed per dst tile with
a one-hot matmul: S[e, d] = (iota == dstrel[e]) * ew[e], psum += S.T @ M.
Layer 1 (800K edges, h0 -> out[25000,128]): src-sharded. Core c takes edges
whose src lies in its own h0 slice, computes partial sums over all 196 dst
tiles; the host sums the 8 partials.

SPMD constraint: one program for all cores, so per-(chunk, tile) group counts
are max'd across cores and padded with zero-weight edges (idx 0, ew 0).
"""
import os
import sys
import time

sys.path.insert(0, "/opt/trn_rl_repo")

import numpy as np
import ml_dtypes

import concourse.bacc as bacc
import concourse.mybir as mybir
from concourse import tile
from concourse.bass_utils import run_bass_kernel_spmd

BF16 = mybir.dt.bfloat16
F32 = mybir.dt.float32
I16 = mybir.dt.int16
I32 = mybir.dt.int32

N_SRC0, N_DST0, N_DST1 = 100000, 50000, 25000
D = 128
NCORES = 8
T0 = 49            # dst tiles per core, layer 0 (49*128*8 = 50176 >= 50000)
SLICE0 = T0 * 128  # 6272 dst rows per core
NCHUNK = 4         # x row chunks for int16 gather indices
CHUNK = 25000
T1 = 196           # dst tiles, layer 1 (196*128 = 25088 >= 25000)
CALL_G = 8         # gather-call size in 128-edge groups (1024 indices; larger calls crash the device)

_last_results = None
_last_nc = None


def _wrap_idx_calls(idx_stream, spans):
    blocks = np.zeros((len(spans), 128, CALL_G * 8), np.int16)
    for i, (ck, a, b) in enumerate(spans):
        seg = idx_stream[a * 128 : b * 128]
        w = np.ascontiguousarray(seg.reshape(-1, 16).T)
        blocks[i, :, : w.shape[1]] = np.tile(w, (8, 1))
    return blocks


def _pack_core(src, dst_local, w, nslot, key, G, slot_base, chunk_of_slot, chunk_size):
    """Pack one core's edges into the padded stream layout.

    Returns (idx_stream int16, dr f32 [128,Gtot], ew f32 [128,Gtot])."""
    order = np.argsort(key, kind="stable")
    src, dstl, w, key = src[order], dst_local[order], w[order], key[order]
    n = len(key)
    run_start = np.searchsorted(key, np.arange(nslot))
    rank = np.arange(n) - run_start[key]
    pos = slot_base[key] * 128 + rank

    Gtot = int(G.sum())
    Ep = Gtot * 128
    idx_stream = np.zeros(Ep, np.int16)
    dr_stream = np.zeros(Ep, np.float32)
    ew_stream = np.zeros(Ep, np.float32)
    local_idx = src - chunk_of_slot[key] * chunk_size
    idx_stream[pos] = local_idx.astype(np.int16)
    dr_stream[pos] = (dstl % 128).astype(np.float32)
    ew_stream[pos] = w
    dr_m = np.ascontiguousarray(dr_stream.reshape(Gtot, 128).T)
    ew_m = np.ascontiguousarray(ew_stream.reshape(Gtot, 128).T)
    return idx_stream, dr_m, ew_m


def _build_program(G0, spans0, G1, spans1):
    """Build the SPMD Bass program. G0: [NCHUNK,T0] int group counts,
    spans0: list of (chunk, gstart, gend) gather-call spans (global group idx),
    G1: [T1], spans1: list of (gstart, gend)."""
    G0tot = int(G0.sum())
    G1tot = int(G1.sum())
    nc = bacc.Bacc("TRN2", target_bir_lowering=False, debug=False,
                   num_devices=NCORES)
    x_d = nc.dram_tensor("x", [N_SRC0, D], BF16, kind="ExternalInput")
    idxs0_d = nc.dram_tensor("idxs0", [len(spans0), 128, CALL_G * 8], I16,
                             kind="ExternalInput")
    dr0_d = nc.dram_tensor("dr0", [128, G0tot], F32, kind="ExternalInput")
    ew0_d = nc.dram_tensor("ew0", [128, G0tot], F32, kind="ExternalInput")
    idxs1_d = nc.dram_tensor("idxs1", [len(spans1), 128, CALL_G * 8], I16,
                             kind="ExternalInput")
    dr1_d = nc.dram_tensor("dr1", [128, G1tot], F32, kind="ExternalInput")
    ew1_d = nc.dram_tensor("ew1", [128, G1tot], F32, kind="ExternalInput")
    h0_d = nc.dram_tensor("h0", [SLICE0, D], BF16)
    out_d = nc.dram_tensor("part", [T1 * 128, D], F32, kind="ExternalOutput")

    with tile.TileContext(nc) as tc:
        with (
            tc.tile_pool(name="const", bufs=1) as cpool,
            tc.tile_pool(name="mpool", bufs=3) as mpool,
            tc.tile_pool(name="ipool", bufs=3) as ipool,
            tc.tile_pool(name="spool", bufs=4) as spool,
            tc.tile_pool(name="opool", bufs=3) as opool,
            tc.tile_pool(name="psum", bufs=4, space="PSUM") as ppool,
        ):
            iota32 = cpool.tile([128, 128], I32)
            iotabf = cpool.tile([128, 128], BF16)
            nc.gpsimd.iota(iota32[:], pattern=[[1, 128]], base=0,
                           channel_multiplier=0)
            nc.vector.tensor_copy(iotabf[:], iota32[:])

            dr0 = cpool.tile([128, G0tot], F32)
            ew0 = cpool.tile([128, G0tot], F32)
            dr1 = cpool.tile([128, G1tot], F32)
            ew1 = cpool.tile([128, G1tot], F32)
            nc.sync.dma_start(dr0[:], dr0_d[:])
            nc.sync.dma_start(ew0[:], ew0_d[:])
            nc.sync.dma_start(dr1[:], dr1_d[:])
            nc.sync.dma_start(ew1[:], ew1_d[:])

            h0acc = cpool.tile([128, T0 * 128], F32)

            def emit_layer(spans, idxs_dram, table_ap, chunk_starts, Gtab,
                           dr_t, ew_t, slot_sink):
                """Walk groups in global order; emit gather calls per span and
                matmul chains per slot. slot_sink(slot, psum_tile) consumes a
                finished psum tile."""
                nspans = int(os.environ.get("KB_NSPANS", "0")) or len(spans)
                span_i = -1
                mtile = None
                span_a = span_b = 0
                g = 0
                gmax = spans[nspans - 1][2]
                for slot, Gs in enumerate(Gtab):
                    if Gs == 0 or g + Gs > gmax:
                        continue
                    psum = ppool.tile([128, 128], F32)
                    for j in range(Gs):
                        if g >= span_b:
                            span_i += 1
                            ck, a, b = spans[span_i]
                            span_a, span_b = a, b
                            L = (b - a) * 128
                            it = ipool.tile([128, CALL_G * 8], I16)
                            nc.sync.dma_start(it[:], idxs_dram[span_i])
                            mtile = mpool.tile([128, CALL_G, 128], BF16)
                            nc.gpsimd.dma_gather(
                                mtile[:, : b - a, :],
                                table_ap[ck],
                                it[:, : L // 16],
                                num_idxs=L,
                                num_idxs_reg=L,
                                elem_size=128,
                            )
                        S = spool.tile([128, 128], BF16)
                        nc.vector.tensor_scalar(
                            S[:], iotabf[:], dr_t[:, g : g + 1],
                            ew_t[:, g : g + 1],
                            mybir.AluOpType.is_equal, mybir.AluOpType.mult,
                        )
                        nc.tensor.matmul(
                            psum[:], S[:], mtile[:, g - span_a, :],
                            start=(j == 0), stop=(j == Gs - 1),
                        )
                        g += 1
                    slot_sink(slot, psum)

            skip_l0 = bool(int(os.environ.get("KB_SKIP_L0", "0")))
            skip_l1 = bool(int(os.environ.get("KB_SKIP_L1", "0")))
            # ---- layer 0 ----
            x_chunks = [x_d[k * CHUNK : (k + 1) * CHUNK, :] for k in range(NCHUNK)]

            def sink0(slot, psum):
                k, t = divmod(slot, T0)
                blk = h0acc[:, t * 128 : (t + 1) * 128]
                if k == 0:
                    nc.vector.tensor_copy(blk, psum[:])
                else:
                    nc.vector.tensor_tensor(blk, psum[:], blk,
                                            mybir.AluOpType.add)

            if skip_l0:
                nc.vector.memset(h0acc[:], 0.0)
            else:
                emit_layer(spans0, idxs0_d, x_chunks, None, G0.reshape(-1),
                           dr0, ew0, sink0)

            for t in range(T0):
                h0bf_t = opool.tile([128, 128], BF16, tag="h0bf")
                nc.vector.tensor_copy(h0bf_t[:],
                                      h0acc[:, t * 128 : (t + 1) * 128])
                nc.sync.dma_start(h0_d[t * 128 : (t + 1) * 128, :], h0bf_t[:])

            # ---- layer 1 ----
            def sink1(t, psum):
                ob = opool.tile([128, 128], F32)
                nc.vector.tensor_copy(ob[:], psum[:])
                nc.sync.dma_start(out_d[t * 128 : (t + 1) * 128, :], ob[:])

            if skip_l1:
                zb = opool.tile([128, 128], F32, tag="zb")
                nc.vector.memset(zb[:], 0.0)
                for t in range(T1):
                    nc.sync.dma_start(out_d[t * 128 : (t + 1) * 128, :], zb[:])
            else:
                emit_layer(spans1, idxs1_d, [h0_d[:]], None, G1, dr1, ew1,
                           sink1)

    nc.compile()
    return nc


def kernel(x, src0, dst0, ew0, src1, dst1, ew1, n_dst0, n_dst1):
    global _last_results
    t_start = time.time()
    x = np.asarray(x, dtype=np.float32)
    src0 = np.asarray(src0).astype(np.int64)
    dst0 = np.asarray(dst0).astype(np.int64)
    ew0 = np.asarray(ew0, dtype=np.float32)
    src1 = np.asarray(src1).astype(np.int64)
    dst1 = np.asarray(dst1).astype(np.int64)
    ew1 = np.asarray(ew1, dtype=np.float32)

    x_bf = x.astype(ml_dtypes.bfloat16)

    # ---- per-core selection + counts ----
    core0 = dst0 // SLICE0          # L0: dst-sharded
    core1 = src1 // SLICE0          # L1: src-sharded (matches h0 slice)
    key0_all = (src0 // CHUNK) * T0 + (dst0 % SLICE0) // 128
    key1_all = dst1 // 128

    cnt0 = np.zeros((NCORES, NCHUNK * T0), np.int64)
    cnt1 = np.zeros((NCORES, T1), np.int64)
    sel0 = [core0 == c for c in range(NCORES)]
    sel1 = [core1 == c for c in range(NCORES)]
    for c in range(NCORES):
        cnt0[c] = np.bincount(key0_all[sel0[c]], minlength=NCHUNK * T0)
        cnt1[c] = np.bincount(key1_all[sel1[c]], minlength=T1)

    G0 = -(-cnt0.max(axis=0) // 128)          # ceil div, per (chunk, tile)
    G0 = G0.reshape(NCHUNK, T0)
    G0[0] = np.maximum(G0[0], 1)              # ensure k=0 init-copy per tile
    G1 = np.maximum(-(-cnt1.max(axis=0) // 128), 1)

    # gather-call spans (global group indices), per chunk for L0
    slot_base0 = np.concatenate([[0], np.cumsum(G0.reshape(-1))[:-1]])
    spans0 = []
    goff = 0
    for k in range(NCHUNK):
        r = int(G0[k].sum())
        a = goff
        while a < goff + r:
            b = min(a + CALL_G, goff + r)
            spans0.append((k, a, b))
            a = b
        goff += r
    slot_base1 = np.concatenate([[0], np.cumsum(G1)[:-1]])
    spans1 = []
    a = 0
    g1tot = int(G1.sum())
    while a < g1tot:
        b = min(a + CALL_G, g1tot)
        spans1.append((0, a, b))
        a = b

    # ---- per-core packed data ----
    chunk_of_slot0 = np.repeat(np.arange(NCHUNK), T0)
    in_maps = []
    for c in range(NCORES):
        m0 = sel0[c]
        i0, d0m, e0m = _pack_core(
            src0[m0], dst0[m0] - c * SLICE0, ew0[m0], NCHUNK * T0,
            key0_all[m0], G0.reshape(-1), slot_base0, chunk_of_slot0, CHUNK)
        m1 = sel1[c]
        i1, d1m, e1m = _pack_core(
            src1[m1] - c * SLICE0, dst1[m1], ew1[m1], T1,
            key1_all[m1], G1, slot_base1, np.zeros(T1, np.int64), SLICE0)
        in_maps.append(dict(x=np.asarray(x_bf),
                            idxs0=_wrap_idx_calls(i0, spans0),
                            dr0=d0m, ew0=e0m,
                            idxs1=_wrap_idx_calls(i1, spans1),
                            dr1=d1m, ew1=e1m))
    t_pack = time.time()

    global _last_nc
    nc = _build_program(G0, spans0, G1, spans1)
    _last_nc = nc
    t_build = time.time()

    trace = bool(int(os.environ.get("KBENCH_TRACE", "0")))
    try:
        res = run_bass_kernel_spmd(nc, in_maps, list(range(NCORES)), trace=trace)
    except ModuleNotFoundError:
        res = run_bass_kernel_spmd(nc, in_maps, list(range(NCORES)), trace=False)
    _last_results = res
    t_run = time.time()
    for _ in range(int(os.environ.get("KBENCH_REPEAT", "0"))):
        t_r = time.time()
        res = run_bass_kernel_spmd(nc, in_maps, list(range(NCORES)), trace=False)
        print(f"[kernel] repeat run wall {time.time() - t_r:.3f}s",
              file=sys.stderr)
    print(f"[kernel] pack {t_pack - t_start:.1f}s build+compile "
          f"{t_build - t_pack:.1f}s run {t_run - t_build:.1f}s",
          file=sys.stderr)

    out = np.zeros((T1 * 128, D), np.float32)
    for c in range(NCORES):
        out += res.results[c]["part"]
    return out[: int(n_dst1)]



# revision 15
# speedup vs baseline: 1.4505x; 1.4505x over previous
"""LightGCN 2-layer propagation on 8 TRN2 NeuronCores.

Layer 0 (1.6M edges, x[100000,128] -> h0[50000,128]): dst-sharded. Core c owns
49 dst tiles of 128 rows. Edges are grouped into slots of paired dst tiles per
x-chunk (4 chunks of 25000 rows so gather indices fit int16), sorted by dst.
Edge source rows are gathered (bf16) via GPSIMD dma_gather; a one-hot matrix
S[e, d] = (iota == dst_rel[e]) * ew[e] is built per 128-edge group (on DVE via
tensor_scalar, a fraction on ACT via Square+Relu), and psum += S.T @ M
accumulates per dst tile. PSUM banks hold 4 dst tiles each (quarter regions,
one accumulation chain per bank), letting a tile integrate all 4 chunks in one
chain: L0 runs as 2 passes (28 + 21 tiles) over the 4 chunks. ACT copies psum
quarters into a bf16 h0 buffer, stored to DRAM with one rearranged DMA per
pass.
Layer 1 (800K edges, h0 -> out[25000,128]): src-sharded. Core c takes edges
whose src lies in its own h0 slice, gathers from its h0, accumulates over all
196 dst tiles (paired slots, 8-bank rotation), stages 28 tiles per f32 buffer
and stores 7 blocks; the host sums the 8 partial outputs.

SPMD: one program for all cores. Per-slot group counts are max'd across cores;
slack edges are padded (idx 0, dst sentinel -1, ew 0). Where a 128-edge group
straddles both tiles of a slot on any core, S is built per (group, tile) with
dr relative to that tile (out-of-tile edges never match iota, so S rows are 0).
"""
import os
import sys
import time

sys.path.insert(0, "/opt/trn_rl_repo")

import numpy as np
import ml_dtypes

import concourse.bacc as bacc
import concourse.mybir as mybir
from concourse import tile
from concourse.bass_utils import run_bass_kernel_spmd

BF16 = mybir.dt.bfloat16
F32 = mybir.dt.float32
I16 = mybir.dt.int16
I32 = mybir.dt.int32
AF = mybir.ActivationFunctionType

N_SRC0, N_DST0, N_DST1 = 100000, 50000, 25000
D = 128
NCORES = 8
T0 = 49            # dst tiles per core, layer 0
SLICE0 = T0 * 128  # 6272 dst rows per core
NCHUNK = 4
CHUNK = 25000
T1 = 196           # dst tiles, layer 1
CALL_G = 8         # gather-call size in 128-edge groups (1024 indices)
PASS0 = 28         # L0 pass-0 tiles (7 psum banks); pass 1 gets 21
ACTK0 = int(os.environ.get("KB_ACTK0", "8"))   # L0: every k-th S build on ACT
ACTK1 = int(os.environ.get("KB_ACTK1", "12"))  # L1: every k-th S build on ACT
STAGE_T = 28       # L1 out tiles per staging buffer (196 = 7*28)

_last_results = None
_last_nc = None


def _slot_pairs(tiles):
    out = []
    i = 0
    while i < len(tiles):
        out.append(tuple(tiles[i : i + 2]))
        i += 2
    return out


class _Packer:
    """Accumulates the SPMD program structure + per-core data streams."""

    def __init__(self):
        self.idx_cols = []      # per core: list of [128, L*8] int16 blocks
        self.builds = []        # program: (layer, g_global, tile, engine)
        self.drv = []           # per core: list of len-128 f32 cols (DVE)
        self.ewv = []
        self.drna = []          # per core: ACT cols (-dr, ew, -ew)
        self.ewa = []
        self.ewna = []
        self.spans = []         # program: (layer, table, gstart, gend)
        self.g_total = 0
        self.build_no = 0

    def pack_layer(self, layer, runs, idx_all, dst_local_all, ew_all, sel_runs):
        """runs: list of (table_id, [slot tuples of tile ids]).
        sel_runs[(r, c)] -> bool mask of core c's edges for run r.
        idx_all/dst_local_all/ew_all: per-core arrays aligned with the masks.
        Returns program info: list of per-run group ranges + build entries.
        """
        prog = []
        for ri, (table, slots) in enumerate(runs):
            run_g0 = self.g_total
            for slot in slots:
                per_core = []
                for c in range(NCORES):
                    m = sel_runs[(ri, c)]
                    dl = dst_local_all[c][m]
                    tsel = dl // 128
                    smask = np.isin(tsel, slot)
                    order = np.argsort(dl[smask], kind="stable")
                    per_core.append((idx_all[c][m][smask][order],
                                     dl[smask][order],
                                     ew_all[c][m][smask][order]))
                n = np.array([len(p[0]) for p in per_core])
                t0 = slot[0]
                n0 = np.array([int((p[1] < (t0 + 1) * 128).sum())
                               for p in per_core])
                gs = max(1, -(-int(n.max()) // 128))
                # pad each core to gs*128
                for c in range(NCORES):
                    pad = gs * 128 - n[c]
                    ii = np.concatenate([per_core[c][0],
                                         np.zeros(pad, np.int64)])
                    dd = np.concatenate([per_core[c][1],
                                         np.full(pad, -1, np.int64)])
                    ee = np.concatenate([per_core[c][2],
                                         np.zeros(pad, np.float32)])
                    per_core[c] = (ii, dd, ee)
                # build schedule; every tile of the slot must get >= 1 build
                # so its psum quarter is written and copied.
                if len(slot) == 1:
                    sched = [[(g, t0)] for g in range(gs)]
                elif int((n - n0).max()) == 0:
                    sched = [[(g, t0)] for g in range(gs)]
                    sched[gs - 1].append((gs - 1, slot[1]))
                else:
                    t1 = slot[1]
                    t0_end = max(1, -(-int(n0.max()) // 128))
                    t1_start = int((n0 // 128).min())
                    sched = []
                    for g in range(gs):
                        items = []
                        if g < t0_end:
                            items.append((g, t0))
                        if g >= t1_start:
                            items.append((g, t1))
                        if not items:
                            items.append((g, t0))
                        sched.append(items)
                # emit
                for g in range(gs):
                    gg = self.g_total + g
                    for (g_, t) in sched[g]:
                        actk = ACTK0 if layer == 0 else ACTK1
                        eng = 'A' if (actk > 0 and
                                      self.build_no % actk == actk - 1) \
                            else 'V'
                        self.build_no += 1
                        col_idx = []
                        for c in range(NCORES):
                            dd = per_core[c][1][g * 128:(g + 1) * 128]
                            ee = per_core[c][2][g * 128:(g + 1) * 128]
                            rel = (dd - t * 128).astype(np.float32)
                            if eng == 'V':
                                if c == 0:
                                    self.drv.append([])
                                    self.ewv.append([])
                                self.drv[-1].append(rel)
                                self.ewv[-1].append(ee.astype(np.float32))
                            else:
                                if c == 0:
                                    self.drna.append([])
                                    self.ewa.append([])
                                    self.ewna.append([])
                                self.drna[-1].append(-rel)
                                self.ewa[-1].append(ee.astype(np.float32))
                                self.ewna[-1].append(-ee.astype(np.float32))
                        self.builds.append((layer, gg, t, eng))
                    # idx stream for this group, per core
                    for c in range(NCORES):
                        ii = per_core[c][0][g * 128:(g + 1) * 128]
                        if c == 0:
                            self.idx_cols.append([])
                        w = np.ascontiguousarray(
                            ii.astype(np.int16).reshape(-1, 16).T)
                        self.idx_cols[-1].append(np.tile(w, (8, 1)))
                self.g_total += gs
            prog.append((table, run_g0, self.g_total))
        return prog


def _pack(x_bf, src0, dst0, ew0, src1, dst1, ew1):
    pk = _Packer()

    # ---- balanced L0 tile->core assignment ----
    # Global dst tiles are assigned to (core, local slot) so that the 8 tiles
    # sharing a slot have similar edge counts: SPMD group counts are maxima
    # across cores, so similar counts minimize padding.
    NGT = T0 * NCORES                    # 392 slots; tile 391 is empty pad
    gt0 = dst0 // 128
    cnt_t = np.bincount(gt0, minlength=NGT)
    order = np.argsort(-cnt_t, kind="stable")
    singles = order[-NCORES:]
    rest = order[:-NCORES]
    pairs = rest.reshape(-1, 2)
    porder = pairs[np.argsort(-cnt_t[pairs].sum(1), kind="stable")]
    core_of = np.zeros(NGT, np.int64)
    local_of = np.zeros(NGT, np.int64)
    nslot = len(porder) // NCORES        # 24 pair slots per core
    for s in range(nslot):
        for c in range(NCORES):
            a, b = porder[NCORES * s + c]
            core_of[a] = c
            local_of[a] = 2 * s
            core_of[b] = c
            local_of[b] = 2 * s + 1
    for c in range(NCORES):
        core_of[singles[c]] = c
        local_of[singles[c]] = T0 - 1

    # ---- layer 0 selection ----
    core0 = core_of[gt0]
    chunk0 = src0 // CHUNK
    dst_local0 = local_of[gt0] * 128 + dst0 % 128
    pass_tiles = [list(range(0, PASS0)), list(range(PASS0, T0))]
    runs0 = []
    sel0 = {}
    idx0_all, dl0_all, ew0_all = [], [], []
    for c in range(NCORES):
        m = core0 == c
        idx0_all.append((src0[m] % CHUNK))
        dl0_all.append(dst_local0[m])
        ew0_all.append(ew0[m])
        sel0_chunk = chunk0[m]
        sel0[c] = sel0_chunk
    ri = 0
    sel_runs0 = {}
    for p in range(2):
        tset = set(pass_tiles[p])
        for k in range(NCHUNK):
            slots = _slot_pairs(pass_tiles[p])
            runs0.append((k, slots))
            for c in range(NCORES):
                tl = dl0_all[c] // 128
                sel_runs0[(ri, c)] = (sel0[c] == k) & np.isin(
                    tl, pass_tiles[p])
            ri += 1
    prog0 = pk.pack_layer(0, runs0, idx0_all, dl0_all, ew0_all, sel_runs0)

    # ---- layer 1 selection (src rows follow the L0 tile permutation) ----
    gt1 = src1 // 128
    core1 = core_of[gt1]
    src1_local = local_of[gt1] * 128 + src1 % 128
    idx1_all, dl1_all, ew1_all = [], [], []
    for c in range(NCORES):
        m = core1 == c
        idx1_all.append(src1_local[m])
        dl1_all.append(dst1[m])
        ew1_all.append(ew1[m])
    runs1 = [(0, _slot_pairs(list(range(T1))))]
    sel_runs1 = {}
    for c in range(NCORES):
        sel_runs1[(0, c)] = np.ones(len(idx1_all[c]), bool)
    prog1 = pk.pack_layer(1, runs1, idx1_all, dl1_all, ew1_all, sel_runs1)

    # ---- spans (gather calls) ----
    spans = []
    for layer, prog in ((0, prog0), (1, prog1)):
        for (table, a, b) in prog:
            g = a
            while g < b:
                e = min(g + CALL_G, b)
                spans.append((layer, table, g, e))
                g = e

    # ---- assemble per-core arrays ----
    nv = len(pk.drv)
    na = len(pk.drna)
    in_maps = []
    for c in range(NCORES):
        idxbuf = np.concatenate([blk[c] for blk in pk.idx_cols], axis=1) \
            if pk.idx_cols else np.zeros((128, 0), np.int16)
        drv = np.stack([col[c] for col in pk.drv], axis=1) if nv else \
            np.zeros((128, 0), np.float32)
        ewv = np.stack([col[c] for col in pk.ewv], axis=1) if nv else \
            np.zeros((128, 0), np.float32)
        drna = np.stack([col[c] for col in pk.drna], axis=1) if na else \
            np.zeros((128, 1), np.float32)
        ewa = np.stack([col[c] for col in pk.ewa], axis=1) if na else \
            np.zeros((128, 1), np.float32)
        ewna = np.stack([col[c] for col in pk.ewna], axis=1) if na else \
            np.zeros((128, 1), np.float32)
        in_maps.append(dict(x=np.asarray(x_bf), idxs=idxbuf, drv=drv, ewv=ewv,
                            drna=drna, ewa=ewa, ewna=ewna))
    return pk, spans, in_maps


def _build_program(pk, spans, nv, na, nidxcol):
    builds = pk.builds
    g_total = pk.g_total
    # group -> (span index, col in span)
    g_span = {}
    span_icol = []          # idx-col offset of each span
    off = 0
    for si, (layer, table, a, b) in enumerate(spans):
        span_icol.append(off)
        for g in range(a, b):
            g_span[g] = (si, g - a)
        off += (b - a) * 8

    # bank/quarter assignment + start/stop
    def bank_info(layer, t):
        if layer == 0:
            p = 0 if t < PASS0 else 1
            lt = t - (0 if p == 0 else PASS0)
            return (0, p, lt // 4), lt % 4
        else:
            return (1, t // 32, (t // 4) % 8), t % 4
    first_b = {}
    last_b = {}
    for i, (layer, g, t, eng) in enumerate(builds):
        key, q = bank_info(layer, t)
        first_b.setdefault(key, i)
        last_b[key] = i

    nc = bacc.Bacc("TRN2", target_bir_lowering=False, debug=False,
                   num_devices=NCORES)
    x_d = nc.dram_tensor("x", [N_SRC0, D], BF16, kind="ExternalInput")
    idxs_d = nc.dram_tensor("idxs", [128, nidxcol], I16, kind="ExternalInput")
    drv_d = nc.dram_tensor("drv", [128, max(nv, 1)], F32, kind="ExternalInput")
    ewv_d = nc.dram_tensor("ewv", [128, max(nv, 1)], F32, kind="ExternalInput")
    drna_d = nc.dram_tensor("drna", [128, max(na, 1)], F32,
                            kind="ExternalInput")
    ewa_d = nc.dram_tensor("ewa", [128, max(na, 1)], F32,
                           kind="ExternalInput")
    ewna_d = nc.dram_tensor("ewna", [128, max(na, 1)], F32,
                            kind="ExternalInput")
    h0_d = nc.dram_tensor("h0", [SLICE0, D], BF16)
    out_d = nc.dram_tensor("part", [T1 * 128, D], F32, kind="ExternalOutput")

    with tile.TileContext(nc) as tc:
        with (
            tc.tile_pool(name="const", bufs=1) as cpool,
            tc.tile_pool(name="mpool", bufs=6) as mpool,
            tc.tile_pool(name="spool", bufs=12) as spool,
            tc.tile_pool(name="sqpool", bufs=6) as sqpool,
            tc.tile_pool(name="stage", bufs=2) as stpool,
            tc.tile_pool(name="psum", bufs=1, space="PSUM") as ppool,
        ):
            iota32 = cpool.tile([128, 128], I32)
            iotabf = cpool.tile([128, 128], BF16)
            nc.gpsimd.iota(iota32[:], pattern=[[1, 128]], base=0,
                           channel_multiplier=0)
            nc.vector.tensor_copy(iotabf[:], iota32[:])

            idxs = cpool.tile([128, nidxcol], I16)
            drv = cpool.tile([128, max(nv, 1)], F32)
            ewv = cpool.tile([128, max(nv, 1)], F32)
            drna = cpool.tile([128, max(na, 1)], F32)
            ewa = cpool.tile([128, max(na, 1)], F32)
            ewna = cpool.tile([128, max(na, 1)], F32)
            # interleave loads so the first chunk of every array lands early
            NLOAD = 6
            chunks = []
            for t_, d_ in ((idxs, idxs_d), (drv, drv_d), (ewv, ewv_d),
                           (drna, drna_d), (ewa, ewa_d), (ewna, ewna_d)):
                n = t_.shape[1]
                step = -(-n // NLOAD)
                cl = []
                for i in range(NLOAD):
                    a, b = i * step, min((i + 1) * step, n)
                    if a < b:
                        cl.append((t_, d_, a, b))
                chunks.append(cl)
            for i in range(NLOAD):
                for cl in chunks:
                    if i < len(cl):
                        t_, d_, a, b = cl[i]
                        nc.sync.dma_start(t_[:, a:b], d_[:, a:b])

            h0acc = cpool.tile([128, T0 * 128], BF16)

            banks = {}

            def get_bank(key):
                if key not in banks:
                    banks[key] = [ppool.tile([128, 512], F32,
                                             name=f"bk{key[2]}"),
                                  False]
                return banks[key][0]

            # walk builds in order; manage spans/gathers lazily
            mtiles = {}
            vi = 0
            ai = 0

            def ensure_span(si):
                if si in mtiles:
                    return mtiles[si]
                layer, table, a, b = spans[si]
                L = b - a
                mt = mpool.tile([128, CALL_G, 128], BF16, name="mt")
                tbl = x_d[table * CHUNK:(table + 1) * CHUNK, :] if layer == 0 \
                    else h0_d[:]
                ic = span_icol[si]
                nc.gpsimd.dma_gather(
                    mt[:, :L, :], tbl, idxs[:, ic:ic + L * 8],
                    num_idxs=L * 128, num_idxs_reg=L * 128, elem_size=128)
                mtiles.clear()
                mtiles[si] = mt
                return mt

            copy_after = {}     # build index -> list of copy ops
            # L0: whole-bank copies at end of each pass
            lastb_pass = {}
            for i, (layer, g, t, eng) in enumerate(builds):
                if layer == 0:
                    p = 0 if t < PASS0 else 1
                    lastb_pass[p] = i
            for p in (0, 1):
                ntile = PASS0 if p == 0 else T0 - PASS0
                nbank = -(-ntile // 4)
                ops = []
                for b in range(nbank):
                    ncols = min(4, ntile - b * 4) * 128
                    ops.append(('L0', p, b, ncols))
                ops.append(('H0', p))
                copy_after.setdefault(lastb_pass[p], []).extend(ops)
            # L1: half-bank copies per tile pair, at the pair's last build
            lastb_t1 = {}
            for i, (layer, g, t, eng) in enumerate(builds):
                if layer == 1:
                    lastb_t1[t] = i
            for j in range(T1 // 2):
                t0, t1 = 2 * j, 2 * j + 1
                i = max(lastb_t1[t0], lastb_t1[t1])
                copy_after.setdefault(i, []).append(('L1', j))

            stage_tiles = {}

            def do_copies(items):
                for op in items:
                    if op[0] == 'L0':
                        _, p, b, ncols = op
                        base = (0 if p == 0 else PASS0) * 128
                        bk = banks[(0, p, b)][0]
                        a = base + b * 512
                        nc.scalar.activation(
                            h0acc[:, a: a + ncols],
                            bk[:, :ncols], AF.Copy, bias=0.0, scale=1.0)
                        # store this bank's h0 rows immediately so the L1
                        # gather table completes as soon as possible
                        dram = h0_d[a:a + ncols, :].rearrange(
                            "(t p) d -> p t d", p=128)
                        nc.sync.dma_start(
                            dram, h0acc[:, a:a + ncols].rearrange(
                                "p (t d) -> p t d", d=128))
                    elif op[0] == 'H0':
                        pass
                    else:
                        _, j = op
                        t0 = 2 * j
                        key, q0 = bank_info(1, t0)
                        bk = banks[key][0]
                        jblk = t0 // STAGE_T
                        lt = t0 % STAGE_T
                        if jblk not in stage_tiles:
                            stage_tiles[jblk] = stpool.tile(
                                [128, STAGE_T * 128], F32, name="stg")
                        nc.scalar.activation(
                            stage_tiles[jblk][:, lt * 128:(lt + 2) * 128],
                            bk[:, q0 * 128:(q0 + 2) * 128],
                            AF.Copy, bias=0.0, scale=1.0)
                        if lt == STAGE_T - 2:
                            rows = STAGE_T * 128
                            dram = out_d[jblk * rows:(jblk + 1) * rows, :] \
                                .rearrange("(t p) d -> p t d", p=128)
                            nc.sync.dma_start(
                                dram,
                                stage_tiles[jblk][:].rearrange(
                                    "p (t d) -> p t d", d=128))
                            del stage_tiles[jblk]

            for i, (layer, g, t, eng) in enumerate(builds):
                si, col = g_span[g]
                mt = ensure_span(si)
                key, q = bank_info(layer, t)
                bk = get_bank(key)
                if eng == 'V':
                    S = spool.tile([128, 128], BF16, name="Sv")
                    nc.vector.tensor_scalar(
                        S[:], iotabf[:], drv[:, vi:vi + 1], ewv[:, vi:vi + 1],
                        mybir.AluOpType.is_equal, mybir.AluOpType.mult)
                    vi += 1
                else:
                    sq = sqpool.tile([128, 128], BF16, name="sq")
                    nc.scalar.activation(sq[:], iotabf[:], AF.Square,
                                         bias=drna[:, ai:ai + 1], scale=1.0)
                    S = spool.tile([128, 128], BF16, name="Sa")
                    nc.scalar.activation(S[:], sq[:], AF.Relu,
                                         bias=ewa[:, ai:ai + 1],
                                         scale=ewna[:, ai:ai + 1])
                    ai += 1
                nc.tensor.matmul(bk[:, q * 128:(q + 1) * 128], S[:],
                                 mt[:, col, :],
                                 start=(first_b[key] == i),
                                 stop=(last_b[key] == i))
                if last_b[key] == i:
                    banks[key][1] = True
                if i in copy_after:
                    do_copies(copy_after[i])

    nc.compile()
    return nc


def kernel(x, src0, dst0, ew0, src1, dst1, ew1, n_dst0, n_dst1):
    global _last_results, _last_nc
    t_start = time.time()
    x = np.asarray(x, dtype=np.float32)
    src0 = np.asarray(src0).astype(np.int64)
    dst0 = np.asarray(dst0).astype(np.int64)
    ew0 = np.asarray(ew0, dtype=np.float32)
    src1 = np.asarray(src1).astype(np.int64)
    dst1 = np.asarray(dst1).astype(np.int64)
    ew1 = np.asarray(ew1, dtype=np.float32)

    x_bf = x.astype(ml_dtypes.bfloat16)

    pk, spans, in_maps = _pack(x_bf, src0, dst0, ew0, src1, dst1, ew1)
    nv = len(pk.drv)
    na = len(pk.drna)
    nidxcol = in_maps[0]["idxs"].shape[1]
    t_pack = time.time()

    nc = _build_program(pk, spans, nv, na, nidxcol)
    _last_nc = nc
    t_build = time.time()

    trace = bool(int(os.environ.get("KBENCH_TRACE", "0")))
    try:
        res = run_bass_kernel_spmd(nc, in_maps, list(range(NCORES)),
                                   trace=trace)
    except ModuleNotFoundError:
        res = run_bass_kernel_spmd(nc, in_maps, list(range(NCORES)),
                                   trace=False)
    _last_results = res
    t_run = time.time()
    print(f"[kernel] pack {t_pack - t_start:.1f}s build+compile "
          f"{t_build - t_pack:.1f}s run {t_run - t_build:.1f}s "
          f"groups={pk.g_total} builds={len(pk.builds)} nv={nv} na={na}",
          file=sys.stderr)

    out = np.zeros((T1 * 128, D), np.float32)
    for c in range(NCORES):
        out += res.results[c]["part"]
    return out[: int(n_dst1)]


# revision 17
# speedup vs baseline: 1.4747x; 1.0166x over previous
"""LightGCN 2-layer propagation on 8 TRN2 NeuronCores.

Layer 0 (1.6M edges, x[100000,128] -> h0[50000,128]): dst-sharded. Core c owns
49 dst tiles of 128 rows. Edges are grouped into slots of paired dst tiles per
x-chunk (4 chunks of 25000 rows so gather indices fit int16), sorted by dst.
Edge source rows are gathered (bf16) via GPSIMD dma_gather; a one-hot matrix
S[e, d] = (iota == dst_rel[e]) * ew[e] is built per 128-edge group (on DVE via
tensor_scalar, a fraction on ACT via Square+Relu), and psum += S.T @ M
accumulates per dst tile. PSUM banks hold 4 dst tiles each (quarter regions,
one accumulation chain per bank), letting a tile integrate all 4 chunks in one
chain: L0 runs as 2 passes (28 + 21 tiles) over the 4 chunks. ACT copies psum
quarters into a bf16 h0 buffer, stored to DRAM with one rearranged DMA per
pass.
Layer 1 (800K edges, h0 -> out[25000,128]): src-sharded. Core c takes edges
whose src lies in its own h0 slice, gathers from its h0, accumulates over all
196 dst tiles (paired slots, 8-bank rotation), stages 28 tiles per f32 buffer
and stores 7 blocks; the host sums the 8 partial outputs.

SPMD: one program for all cores. Per-slot group counts are max'd across cores;
slack edges are padded (idx 0, dst sentinel -1, ew 0). Where a 128-edge group
straddles both tiles of a slot on any core, S is built per (group, tile) with
dr relative to that tile (out-of-tile edges never match iota, so S rows are 0).
"""
import os
import sys
import time

sys.path.insert(0, "/opt/trn_rl_repo")

import numpy as np
import ml_dtypes

import concourse.bacc as bacc
import concourse.mybir as mybir
from concourse import tile
from concourse.bass_utils import run_bass_kernel_spmd

BF16 = mybir.dt.bfloat16
F32 = mybir.dt.float32
I16 = mybir.dt.int16
I32 = mybir.dt.int32
AF = mybir.ActivationFunctionType

N_SRC0, N_DST0, N_DST1 = 100000, 50000, 25000
D = 128
NCORES = 8
T0 = 49            # dst tiles per core, layer 0
SLICE0 = T0 * 128  # 6272 dst rows per core
NCHUNK = 4
CHUNK = 25000
T1 = 196           # dst tiles, layer 1
CALL_G = 8         # gather-call size in 128-edge groups (1024 indices)
PASS0 = 28         # L0 pass-0 tiles (7 psum banks); pass 1 gets 21
ACTK0 = int(os.environ.get("KB_ACTK0", "8"))   # L0: every k-th S build on ACT
ACTK1 = int(os.environ.get("KB_ACTK1", "12"))  # L1: every k-th S build on ACT
STAGE_T = 28       # L1 out tiles per staging buffer (196 = 7*28)

_last_results = None
_last_nc = None


def _slot_pairs(tiles):
    out = []
    i = 0
    while i < len(tiles):
        out.append(tuple(tiles[i : i + 2]))
        i += 2
    return out


class _Packer:
    """Accumulates the SPMD program structure + per-core data streams."""

    def __init__(self):
        self.idx_cols = []      # per core: list of [128, L*8] int16 blocks
        self.builds = []        # program: (layer, g_global, tile, engine)
        self.drv = []           # per core: list of len-128 f32 cols (DVE)
        self.ewv = []
        self.drna = []          # per core: ACT cols (-dr, ew, -ew)
        self.ewa = []
        self.ewna = []
        self.spans = []         # program: (layer, table, gstart, gend)
        self.g_total = 0
        self.build_no = 0

    def pack_layer(self, layer, runs, idx_all, dst_local_all, ew_all, sel_runs):
        """runs: list of (table_id, [slot tuples of tile ids]).
        sel_runs[(r, c)] -> bool mask of core c's edges for run r.
        idx_all/dst_local_all/ew_all: per-core arrays aligned with the masks.
        Returns program info: list of per-run group ranges + build entries.
        """
        prog = []
        for ri, (table, slots) in enumerate(runs):
            run_g0 = self.g_total
            for slot in slots:
                per_core = []
                for c in range(NCORES):
                    m = sel_runs[(ri, c)]
                    dl = dst_local_all[c][m]
                    tsel = dl // 128
                    smask = np.isin(tsel, slot)
                    order = np.argsort(dl[smask], kind="stable")
                    per_core.append((idx_all[c][m][smask][order],
                                     dl[smask][order],
                                     ew_all[c][m][smask][order]))
                n = np.array([len(p[0]) for p in per_core])
                t0 = slot[0]
                n0 = np.array([int((p[1] < (t0 + 1) * 128).sum())
                               for p in per_core])
                gs = max(1, -(-int(n.max()) // 128))
                # pad each core to gs*128
                for c in range(NCORES):
                    pad = gs * 128 - n[c]
                    ii = np.concatenate([per_core[c][0],
                                         np.zeros(pad, np.int64)])
                    dd = np.concatenate([per_core[c][1],
                                         np.full(pad, -1, np.int64)])
                    ee = np.concatenate([per_core[c][2],
                                         np.zeros(pad, np.float32)])
                    per_core[c] = (ii, dd, ee)
                # build schedule; every tile of the slot must get >= 1 build
                # so its psum quarter is written and copied.
                if len(slot) == 1:
                    sched = [[(g, t0)] for g in range(gs)]
                elif int((n - n0).max()) == 0:
                    sched = [[(g, t0)] for g in range(gs)]
                    sched[gs - 1].append((gs - 1, slot[1]))
                else:
                    t1 = slot[1]
                    t0_end = max(1, -(-int(n0.max()) // 128))
                    t1_start = int((n0 // 128).min())
                    sched = []
                    for g in range(gs):
                        items = []
                        if g < t0_end:
                            items.append((g, t0))
                        if g >= t1_start:
                            items.append((g, t1))
                        if not items:
                            items.append((g, t0))
                        sched.append(items)
                # emit
                for g in range(gs):
                    gg = self.g_total + g
                    for (g_, t) in sched[g]:
                        actk = ACTK0 if layer == 0 else ACTK1
                        eng = 'A' if (actk > 0 and
                                      self.build_no % actk == actk - 1) \
                            else 'V'
                        self.build_no += 1
                        col_idx = []
                        for c in range(NCORES):
                            dd = per_core[c][1][g * 128:(g + 1) * 128]
                            ee = per_core[c][2][g * 128:(g + 1) * 128]
                            rel = (dd - t * 128).astype(np.float32)
                            if eng == 'V':
                                if c == 0:
                                    self.drv.append([])
                                    self.ewv.append([])
                                self.drv[-1].append(rel)
                                self.ewv[-1].append(ee.astype(np.float32))
                            else:
                                if c == 0:
                                    self.drna.append([])
                                    self.ewa.append([])
                                    self.ewna.append([])
                                self.drna[-1].append(-rel)
                                self.ewa[-1].append(ee.astype(np.float32))
                                self.ewna[-1].append(-ee.astype(np.float32))
                        self.builds.append((layer, gg, t, eng))
                    # idx stream for this group, per core
                    for c in range(NCORES):
                        ii = per_core[c][0][g * 128:(g + 1) * 128]
                        if c == 0:
                            self.idx_cols.append([])
                        w = np.ascontiguousarray(
                            ii.astype(np.int16).reshape(-1, 16).T)
                        self.idx_cols[-1].append(np.tile(w, (8, 1)))
                self.g_total += gs
            prog.append((table, run_g0, self.g_total))
        return prog


def _pack(x_bf, src0, dst0, ew0, src1, dst1, ew1):
    pk = _Packer()

    # ---- balanced L0 tile->core assignment ----
    # Global dst tiles are assigned to (core, local slot) so that the 8 tiles
    # sharing a slot have similar edge counts: SPMD group counts are maxima
    # across cores, so similar counts minimize padding.
    NGT = T0 * NCORES                    # 392 slots; tile 391 is empty pad
    gt0 = dst0 // 128
    cnt_t = np.bincount(gt0, minlength=NGT)
    order = np.argsort(-cnt_t, kind="stable")
    singles = order[-NCORES:]
    rest = order[:-NCORES]
    pairs = rest.reshape(-1, 2)
    porder = pairs[np.argsort(-cnt_t[pairs].sum(1), kind="stable")]
    core_of = np.zeros(NGT, np.int64)
    local_of = np.zeros(NGT, np.int64)
    nslot = len(porder) // NCORES        # 24 pair slots per core
    for s in range(nslot):
        for c in range(NCORES):
            a, b = porder[NCORES * s + c]
            core_of[a] = c
            local_of[a] = 2 * s
            core_of[b] = c
            local_of[b] = 2 * s + 1
    for c in range(NCORES):
        core_of[singles[c]] = c
        local_of[singles[c]] = T0 - 1

    # ---- layer 0 selection ----
    core0 = core_of[gt0]
    chunk0 = src0 // CHUNK
    dst_local0 = local_of[gt0] * 128 + dst0 % 128
    pass_tiles = [list(range(0, PASS0)), list(range(PASS0, T0))]
    runs0 = []
    sel0 = {}
    idx0_all, dl0_all, ew0_all = [], [], []
    for c in range(NCORES):
        m = core0 == c
        idx0_all.append((src0[m] % CHUNK))
        dl0_all.append(dst_local0[m])
        ew0_all.append(ew0[m])
        sel0_chunk = chunk0[m]
        sel0[c] = sel0_chunk
    ri = 0
    sel_runs0 = {}
    for p in range(2):
        tset = set(pass_tiles[p])
        for k in range(NCHUNK):
            slots = _slot_pairs(pass_tiles[p])
            runs0.append((k, slots))
            for c in range(NCORES):
                tl = dl0_all[c] // 128
                sel_runs0[(ri, c)] = (sel0[c] == k) & np.isin(
                    tl, pass_tiles[p])
            ri += 1
    prog0 = pk.pack_layer(0, runs0, idx0_all, dl0_all, ew0_all, sel_runs0)

    # ---- layer 1 selection (src rows follow the L0 tile permutation) ----
    gt1 = src1 // 128
    core1 = core_of[gt1]
    src1_local = local_of[gt1] * 128 + src1 % 128
    idx1_all, dl1_all, ew1_all = [], [], []
    for c in range(NCORES):
        m = core1 == c
        idx1_all.append(src1_local[m])
        dl1_all.append(dst1[m])
        ew1_all.append(ew1[m])
    runs1 = [(0, _slot_pairs(list(range(T1))))]
    sel_runs1 = {}
    for c in range(NCORES):
        sel_runs1[(0, c)] = np.ones(len(idx1_all[c]), bool)
    prog1 = pk.pack_layer(1, runs1, idx1_all, dl1_all, ew1_all, sel_runs1)

    # ---- spans (gather calls) ----
    spans = []
    for layer, prog in ((0, prog0), (1, prog1)):
        for (table, a, b) in prog:
            g = a
            while g < b:
                e = min(g + CALL_G, b)
                spans.append((layer, table, g, e))
                g = e

    # ---- assemble per-core arrays ----
    nv = len(pk.drv)
    na = len(pk.drna)
    in_maps = []
    for c in range(NCORES):
        idxbuf = np.concatenate([blk[c] for blk in pk.idx_cols], axis=1) \
            if pk.idx_cols else np.zeros((128, 0), np.int16)
        drv = np.stack([col[c] for col in pk.drv], axis=1) if nv else \
            np.zeros((128, 0), np.float32)
        ewv = np.stack([col[c] for col in pk.ewv], axis=1) if nv else \
            np.zeros((128, 0), np.float32)
        drna = np.stack([col[c] for col in pk.drna], axis=1) if na else \
            np.zeros((128, 1), np.float32)
        ewa = np.stack([col[c] for col in pk.ewa], axis=1) if na else \
            np.zeros((128, 1), np.float32)
        ewna = np.stack([col[c] for col in pk.ewna], axis=1) if na else \
            np.zeros((128, 1), np.float32)
        in_maps.append(dict(x=np.asarray(x_bf), idxs=idxbuf, drv=drv, ewv=ewv,
                            drna=drna, ewa=ewa, ewna=ewna))
    return pk, spans, in_maps


def _build_program(pk, spans, nv, na, nidxcol):
    builds = pk.builds
    g_total = pk.g_total
    # group -> (span index, col in span)
    g_span = {}
    span_icol = []          # idx-col offset of each span
    off = 0
    for si, (layer, table, a, b) in enumerate(spans):
        span_icol.append(off)
        for g in range(a, b):
            g_span[g] = (si, g - a)
        off += (b - a) * 8

    # bank/quarter assignment + start/stop
    def bank_info(layer, t):
        if layer == 0:
            p = 0 if t < PASS0 else 1
            lt = t - (0 if p == 0 else PASS0)
            return (0, p, lt // 4), lt % 4
        else:
            return (1, t // 32, (t // 4) % 8), t % 4
    first_b = {}
    last_b = {}
    for i, (layer, g, t, eng) in enumerate(builds):
        key, q = bank_info(layer, t)
        first_b.setdefault(key, i)
        last_b[key] = i

    nc = bacc.Bacc("TRN2", target_bir_lowering=False, debug=False,
                   num_devices=NCORES)
    x_d = nc.dram_tensor("x", [N_SRC0, D], BF16, kind="ExternalInput")
    idxs_d = nc.dram_tensor("idxs", [128, nidxcol], I16, kind="ExternalInput")
    drv_d = nc.dram_tensor("drv", [128, max(nv, 1)], F32, kind="ExternalInput")
    ewv_d = nc.dram_tensor("ewv", [128, max(nv, 1)], F32, kind="ExternalInput")
    drna_d = nc.dram_tensor("drna", [128, max(na, 1)], F32,
                            kind="ExternalInput")
    ewa_d = nc.dram_tensor("ewa", [128, max(na, 1)], F32,
                           kind="ExternalInput")
    ewna_d = nc.dram_tensor("ewna", [128, max(na, 1)], F32,
                            kind="ExternalInput")
    h0_d = nc.dram_tensor("h0", [SLICE0, D], BF16)
    out_d = nc.dram_tensor("part", [T1 * 128, D], F32, kind="ExternalOutput")

    with tile.TileContext(nc) as tc:
        with (
            tc.tile_pool(name="const", bufs=1) as cpool,
            tc.tile_pool(name="mpool", bufs=6) as mpool,
            tc.tile_pool(name="spool", bufs=12) as spool,
            tc.tile_pool(name="sqpool", bufs=6) as sqpool,
            tc.tile_pool(name="stage", bufs=2) as stpool,
            tc.tile_pool(name="psum", bufs=1, space="PSUM") as ppool,
        ):
            iota32 = cpool.tile([128, 128], I32)
            iotabf = cpool.tile([128, 128], BF16)
            nc.gpsimd.iota(iota32[:], pattern=[[1, 128]], base=0,
                           channel_multiplier=0)
            nc.vector.tensor_copy(iotabf[:], iota32[:])

            idxs = cpool.tile([128, nidxcol], I16)
            drv = cpool.tile([128, max(nv, 1)], F32)
            ewv = cpool.tile([128, max(nv, 1)], F32)
            drna = cpool.tile([128, max(na, 1)], F32)
            ewa = cpool.tile([128, max(na, 1)], F32)
            ewna = cpool.tile([128, max(na, 1)], F32)
            # interleave loads so the first chunk of every array lands early;
            # the very first chunk of each array is small to unblock compute
            NLOAD = 6
            chunks = []
            for t_, d_ in ((idxs, idxs_d), (drv, drv_d), (ewv, ewv_d),
                           (drna, drna_d), (ewa, ewa_d), (ewna, ewna_d)):
                n = t_.shape[1]
                first = max(1, n // 24)
                step = -(-(n - first) // (NLOAD - 1))
                cl = [(t_, d_, 0, min(first, n))]
                for i in range(NLOAD - 1):
                    a = first + i * step
                    b = min(first + (i + 1) * step, n)
                    if a < b:
                        cl.append((t_, d_, a, b))
                chunks.append(cl)
            for i in range(NLOAD):
                for cl in chunks:
                    if i < len(cl):
                        t_, d_, a, b = cl[i]
                        nc.sync.dma_start(t_[:, a:b], d_[:, a:b])

            h0acc = cpool.tile([128, T0 * 128], BF16)

            banks = {}

            def get_bank(key):
                if key not in banks:
                    banks[key] = [ppool.tile([128, 512], F32,
                                             name=f"bk{key[2]}"),
                                  False]
                return banks[key][0]

            # walk builds in order; manage spans/gathers lazily
            mtiles = {}
            vi = 0
            ai = 0

            def ensure_span(si):
                if si in mtiles:
                    return mtiles[si]
                layer, table, a, b = spans[si]
                L = b - a
                mt = mpool.tile([128, CALL_G, 128], BF16, name="mt")
                tbl = x_d[table * CHUNK:(table + 1) * CHUNK, :] if layer == 0 \
                    else h0_d[:]
                ic = span_icol[si]
                nc.gpsimd.dma_gather(
                    mt[:, :L, :], tbl, idxs[:, ic:ic + L * 8],
                    num_idxs=L * 128, num_idxs_reg=L * 128, elem_size=128)
                mtiles.clear()
                mtiles[si] = mt
                return mt

            copy_after = {}     # build index -> list of copy ops
            # L0: whole-bank copies at end of each pass
            lastb_pass = {}
            for i, (layer, g, t, eng) in enumerate(builds):
                if layer == 0:
                    p = 0 if t < PASS0 else 1
                    lastb_pass[p] = i
            for p in (0, 1):
                ntile = PASS0 if p == 0 else T0 - PASS0
                nbank = -(-ntile // 4)
                ops = []
                for b in range(nbank):
                    ncols = min(4, ntile - b * 4) * 128
                    ops.append(('L0', p, b, ncols))
                ops.append(('H0', p))
                copy_after.setdefault(lastb_pass[p], []).extend(ops)
            # L1: half-bank copies per tile pair, at the pair's last build
            lastb_t1 = {}
            for i, (layer, g, t, eng) in enumerate(builds):
                if layer == 1:
                    lastb_t1[t] = i
            for j in range(T1 // 2):
                t0, t1 = 2 * j, 2 * j + 1
                i = max(lastb_t1[t0], lastb_t1[t1])
                copy_after.setdefault(i, []).append(('L1', j))

            stage_tiles = {}

            def do_copies(items):
                for op in items:
                    if op[0] == 'L0':
                        _, p, b, ncols = op
                        base = (0 if p == 0 else PASS0) * 128
                        bk = banks[(0, p, b)][0]
                        a = base + b * 512
                        nc.scalar.activation(
                            h0acc[:, a: a + ncols],
                            bk[:, :ncols], AF.Copy, bias=0.0, scale=1.0)
                        # store this bank's h0 rows immediately so the L1
                        # gather table completes as soon as possible
                        dram = h0_d[a:a + ncols, :].rearrange(
                            "(t p) d -> p t d", p=128)
                        nc.sync.dma_start(
                            dram, h0acc[:, a:a + ncols].rearrange(
                                "p (t d) -> p t d", d=128))
                    elif op[0] == 'H0':
                        pass
                    else:
                        _, j = op
                        t0 = 2 * j
                        key, q0 = bank_info(1, t0)
                        bk = banks[key][0]
                        jblk = t0 // STAGE_T
                        lt = t0 % STAGE_T
                        if jblk not in stage_tiles:
                            stage_tiles[jblk] = stpool.tile(
                                [128, STAGE_T * 128], F32, name="stg")
                        nc.scalar.activation(
                            stage_tiles[jblk][:, lt * 128:(lt + 2) * 128],
                            bk[:, q0 * 128:(q0 + 2) * 128],
                            AF.Copy, bias=0.0, scale=1.0)
                        last_blk = jblk == T1 // STAGE_T - 1
                        parts = ((12, 0, 14), (STAGE_T - 2, 14, STAGE_T)) \
                            if last_blk else ((STAGE_T - 2, 0, STAGE_T),)
                        for (trig, c0, c1) in parts:
                            if lt != trig:
                                continue
                            rows = STAGE_T * 128
                            dram = out_d[jblk * rows + c0 * 128:
                                         jblk * rows + c1 * 128, :] \
                                .rearrange("(t p) d -> p t d", p=128)
                            nc.sync.dma_start(
                                dram,
                                stage_tiles[jblk][:, c0 * 128:c1 * 128]
                                .rearrange("p (t d) -> p t d", d=128))
                            if c1 == STAGE_T:
                                del stage_tiles[jblk]

            for i, (layer, g, t, eng) in enumerate(builds):
                si, col = g_span[g]
                mt = ensure_span(si)
                key, q = bank_info(layer, t)
                bk = get_bank(key)
                if eng == 'V':
                    S = spool.tile([128, 128], BF16, name="Sv")
                    nc.vector.tensor_scalar(
                        S[:], iotabf[:], drv[:, vi:vi + 1], ewv[:, vi:vi + 1],
                        mybir.AluOpType.is_equal, mybir.AluOpType.mult)
                    vi += 1
                else:
                    sq = sqpool.tile([128, 128], BF16, name="sq")
                    nc.scalar.activation(sq[:], iotabf[:], AF.Square,
                                         bias=drna[:, ai:ai + 1], scale=1.0)
                    S = spool.tile([128, 128], BF16, name="Sa")
                    nc.scalar.activation(S[:], sq[:], AF.Relu,
                                         bias=ewa[:, ai:ai + 1],
                                         scale=ewna[:, ai:ai + 1])
                    ai += 1
                nc.tensor.matmul(bk[:, q * 128:(q + 1) * 128], S[:],
                                 mt[:, col, :],
                                 start=(first_b[key] == i),
                                 stop=(last_b[key] == i))
                if last_b[key] == i:
                    banks[key][1] = True
                if i in copy_after:
                    do_copies(copy_after[i])

    nc.compile()
    return nc


def kernel(x, src0, dst0, ew0, src1, dst1, ew1, n_dst0, n_dst1):
    global _last_results, _last_nc
    t_start = time.time()
    x = np.asarray(x, dtype=np.float32)
    src0 = np.asarray(src0).astype(np.int64)
    dst0 = np.asarray(dst0).astype(np.int64)
    ew0 = np.asarray(ew0, dtype=np.float32)
    src1 = np.asarray(src1).astype(np.int64)
    dst1 = np.asarray(dst1).astype(np.int64)
    ew1 = np.asarray(ew1, dtype=np.float32)

    x_bf = x.astype(ml_dtypes.bfloat16)

    pk, spans, in_maps = _pack(x_bf, src0, dst0, ew0, src1, dst1, ew1)
    nv = len(pk.drv)
    na = len(pk.drna)
    nidxcol = in_maps[0]["idxs"].shape[1]
    t_pack = time.time()

    nc = _build_program(pk, spans, nv, na, nidxcol)
    _last_nc = nc
    t_build = time.time()

    trace = bool(int(os.environ.get("KBENCH_TRACE", "0")))
    try:
        res = run_bass_kernel_spmd(nc, in_maps, list(range(NCORES)),
                                   trace=trace)
    except ModuleNotFoundError:
        res = run_bass_kernel_spmd(nc, in_maps, list(range(NCORES)),
                                   trace=False)
    _last_results = res
    t_run = time.time()
    print(f"[kernel] pack {t_pack - t_start:.1f}s build+compile "
          f"{t_build - t_pack:.1f}s run {t_run - t_build:.1f}s "
          f"groups={pk.g_total} builds={len(pk.builds)} nv={nv} na={na}",
          file=sys.stderr)

    out = np.zeros((T1 * 128, D), np.float32)
    for c in range(NCORES):
        out += res.results[c]["part"]
    return out[: int(n_dst1)]


# revision 29
# speedup vs baseline: 1.5635x; 1.0603x over previous
"""LightGCN 2-layer propagation on 8 TRN2 NeuronCores.

Layer 0 (1.6M edges, x[100000,128] -> h0[50000,128]): dst-sharded. Core c owns
49 dst tiles of 128 rows. Edges are grouped into slots of paired dst tiles per
x-chunk (4 chunks of 25000 rows so gather indices fit int16), sorted by dst.
Edge source rows are gathered (bf16) via GPSIMD dma_gather; a one-hot matrix
S[e, d] = (iota == dst_rel[e]) * ew[e] is built per 128-edge group (on DVE via
tensor_scalar, a fraction on ACT via Square+Relu), and psum += S.T @ M
accumulates per dst tile. PSUM banks hold 4 dst tiles each (quarter regions,
one accumulation chain per bank), letting a tile integrate all 4 chunks in one
chain: L0 runs as 2 passes (28 + 21 tiles) over the 4 chunks. ACT copies psum
quarters into a bf16 h0 buffer, stored to DRAM with one rearranged DMA per
pass.
Layer 1 (800K edges, h0 -> out[25000,128]): src-sharded. Core c takes edges
whose src lies in its own h0 slice, gathers from its h0, accumulates over all
196 dst tiles (paired slots, 8-bank rotation), stages 28 tiles per f32 buffer
and stores 7 blocks; the host sums the 8 partial outputs.

SPMD: one program for all cores. Per-slot group counts are max'd across cores;
slack edges are padded (idx 0, dst sentinel -1, ew 0). Where a 128-edge group
straddles both tiles of a slot on any core, S is built per (group, tile) with
dr relative to that tile (out-of-tile edges never match iota, so S rows are 0).
"""
import os
import sys
import time

sys.path.insert(0, "/opt/trn_rl_repo")

import numpy as np
import ml_dtypes

import concourse.bacc as bacc
import concourse.mybir as mybir
from concourse import tile
from concourse.bass_utils import run_bass_kernel_spmd

BF16 = mybir.dt.bfloat16
F32 = mybir.dt.float32
I16 = mybir.dt.int16
I32 = mybir.dt.int32
AF = mybir.ActivationFunctionType

N_SRC0, N_DST0, N_DST1 = 100000, 50000, 25000
D = 128
NCORES = 8
T0 = 49            # dst tiles per core, layer 0
SLICE0 = T0 * 128  # 6272 dst rows per core
NCHUNK = 4
CHUNK = 25000
T1 = 196           # dst tiles, layer 1
CALL_G = 8         # gather-call size in 128-edge groups (1024 indices)
PASS0 = 28         # L0 pass-0 tiles (7 psum banks); pass 1 gets 21
ACTK0 = int(os.environ.get("KB_ACTK0", "8"))   # L0: every k-th S build on ACT
ACTK1 = int(os.environ.get("KB_ACTK1", "12"))  # L1: every k-th S build on ACT
STAGE_T = 28       # L1 out tiles per staging buffer (196 = 7*28)

_last_results = None
_last_nc = None


SLOT0 = int(os.environ.get("KB_SLOT0", "14"))  # L0 slot size (tiles)
SLOT1 = int(os.environ.get("KB_SLOT1", "16"))  # L1 slot size (tiles)


def _slot_blocks(tiles, size):
    out = []
    i = 0
    while i < len(tiles):
        out.append(tuple(tiles[i : i + size]))
        i += size
    return out


class _Packer:
    """Accumulates the SPMD program structure + per-core data streams."""

    def __init__(self):
        self.idx_cols = []      # per core: list of [128, L*8] int16 blocks
        self.builds = []        # program: (layer, g_global, tile, engine)
        self.drv = []           # per core: list of len-128 f32 cols (DVE)
        self.ewv = []
        self.drna = []          # per core: ACT cols (-dr, ew, -ew)
        self.ewa = []
        self.ewna = []
        self.spans = []         # program: (layer, table, gstart, gend)
        self.g_total = 0
        self.build_no = 0

    def pack_layer(self, layer, runs, idx_all, dst_local_all, ew_all, sel_runs):
        """runs: list of (table_id, [slot tuples of tile ids]).
        sel_runs[(r, c)] -> bool mask of core c's edges for run r.
        idx_all/dst_local_all/ew_all: per-core arrays aligned with the masks.
        Returns program info: list of per-run group ranges + build entries.
        """
        prog = []
        for ri, (table, slots) in enumerate(runs):
            run_g0 = self.g_total
            for slot in slots:
                per_core = []
                for c in range(NCORES):
                    m = sel_runs[(ri, c)]
                    dl = dst_local_all[c][m]
                    tsel = dl // 128
                    smask = np.isin(tsel, slot)
                    order = np.argsort(dl[smask], kind="stable")
                    per_core.append((idx_all[c][m][smask][order],
                                     dl[smask][order],
                                     ew_all[c][m][smask][order]))
                n = np.array([len(p[0]) for p in per_core])
                gs = max(1, -(-int(n.max()) // 128))
                # cumulative edge counts per tile boundary (slot tiles are
                # contiguous and each core's edges are dst-sorted)
                m_ = len(slot)
                cums = np.zeros((NCORES, m_), np.int64)
                for c in range(NCORES):
                    dl = per_core[c][1]
                    for i_t, t in enumerate(slot):
                        cums[c, i_t] = np.searchsorted(dl, (t + 1) * 128)
                # pad each core to gs*128
                for c in range(NCORES):
                    pad = gs * 128 - n[c]
                    ii = np.concatenate([per_core[c][0],
                                         np.zeros(pad, np.int64)])
                    dd = np.concatenate([per_core[c][1],
                                         np.full(pad, -1, np.int64)])
                    ee = np.concatenate([per_core[c][2],
                                         np.zeros(pad, np.float32)])
                    per_core[c] = (ii, dd, ee)
                # per-tile build window [lo, hi) over groups; every tile gets
                # >= 1 build so its psum quarter is written and copied
                los = []
                his = []
                for i_t in range(m_):
                    lo = 0 if i_t == 0 else int((cums[:, i_t - 1] // 128)
                                                .min())
                    hi = -(-int(cums[:, i_t].max()) // 128)
                    lo = min(lo, gs - 1)
                    hi = min(max(hi, lo + 1), gs)
                    los.append(lo)
                    his.append(hi)
                sched = [[] for _ in range(gs)]
                for i_t, t in enumerate(slot):
                    for g in range(los[i_t], his[i_t]):
                        sched[g].append(t)
                # emit; adjacent-tile builds in a group merge into ONE wide
                # [128,256] S build (dr relative to the lower tile)
                for g in range(gs):
                    gg = self.g_total + g
                    tl = sched[g]
                    items = []
                    j = 0
                    while j < len(tl):
                        if j + 1 < len(tl) and tl[j + 1] == tl[j] + 1:
                            items.append((g, tl[j], 2))
                            j += 2
                        else:
                            items.append((g, tl[j], 1))
                            j += 1
                    for (g_, t, width) in items:
                        actk = ACTK0 if layer == 0 else ACTK1
                        eng = 'A' if (actk > 0 and
                                      self.build_no % actk == actk - 1) \
                            else 'V'
                        self.build_no += 1
                        for c in range(NCORES):
                            dd = per_core[c][1][g * 128:(g + 1) * 128]
                            ee = per_core[c][2][g * 128:(g + 1) * 128]
                            rel = (dd - t * 128).astype(np.float32)
                            if eng == 'V':
                                if c == 0:
                                    self.drv.append([])
                                    self.ewv.append([])
                                self.drv[-1].append(rel)
                                self.ewv[-1].append(ee.astype(np.float32))
                            else:
                                if c == 0:
                                    self.drna.append([])
                                    self.ewa.append([])
                                    self.ewna.append([])
                                self.drna[-1].append(-rel)
                                self.ewa[-1].append(ee.astype(np.float32))
                                self.ewna[-1].append(-ee.astype(np.float32))
                        self.builds.append((layer, gg, t, eng, width))
                    # idx stream for this group, per core
                    for c in range(NCORES):
                        ii = per_core[c][0][g * 128:(g + 1) * 128]
                        if c == 0:
                            self.idx_cols.append([])
                        w = np.ascontiguousarray(
                            ii.astype(np.int16).reshape(-1, 16).T)
                        self.idx_cols[-1].append(np.tile(w, (8, 1)))
                self.g_total += gs
            prog.append((table, run_g0, self.g_total))
        return prog


def _pack(x_bf, src0, dst0, ew0, src1, dst1, ew1):
    pk = _Packer()

    # ---- balanced L0 tile->core assignment ----
    # Global dst tiles are assigned to (core, local slot) so that the 8 tiles
    # sharing a slot have similar edge counts: SPMD group counts are maxima
    # across cores, so similar counts minimize padding.
    NGT = T0 * NCORES                    # 392 slots; tile 391 is empty pad
    gt0 = dst0 // 128
    cnt_t = np.bincount(gt0, minlength=NGT)
    order = np.argsort(-cnt_t, kind="stable")
    singles = order[-NCORES:]
    rest = order[:-NCORES]
    pairs = rest.reshape(-1, 2)
    porder = pairs[np.argsort(-cnt_t[pairs].sum(1), kind="stable")]
    core_of = np.zeros(NGT, np.int64)
    local_of = np.zeros(NGT, np.int64)
    nslot = len(porder) // NCORES        # 24 pair slots per core
    for s in range(nslot):
        for c in range(NCORES):
            a, b = porder[NCORES * s + c]
            core_of[a] = c
            local_of[a] = 2 * s
            core_of[b] = c
            local_of[b] = 2 * s + 1
    for c in range(NCORES):
        core_of[singles[c]] = c
        local_of[singles[c]] = T0 - 1

    # ---- layer 0 selection ----
    core0 = core_of[gt0]
    chunk0 = src0 // CHUNK
    dst_local0 = local_of[gt0] * 128 + dst0 % 128
    pass_tiles = [list(range(0, PASS0)), list(range(PASS0, T0))]
    runs0 = []
    sel0 = {}
    idx0_all, dl0_all, ew0_all = [], [], []
    for c in range(NCORES):
        m = core0 == c
        idx0_all.append((src0[m] % CHUNK))
        dl0_all.append(dst_local0[m])
        ew0_all.append(ew0[m])
        sel0_chunk = chunk0[m]
        sel0[c] = sel0_chunk
    ri = 0
    sel_runs0 = {}
    for p in range(2):
        tset = set(pass_tiles[p])
        for k in range(NCHUNK):
            slots = _slot_blocks(pass_tiles[p], SLOT0)
            runs0.append((k, slots))
            for c in range(NCORES):
                tl = dl0_all[c] // 128
                sel_runs0[(ri, c)] = (sel0[c] == k) & np.isin(
                    tl, pass_tiles[p])
            ri += 1
    prog0 = pk.pack_layer(0, runs0, idx0_all, dl0_all, ew0_all, sel_runs0)

    # ---- layer 1 selection (src rows follow the L0 tile permutation) ----
    gt1 = src1 // 128
    core1 = core_of[gt1]
    src1_local = local_of[gt1] * 128 + src1 % 128
    idx1_all, dl1_all, ew1_all = [], [], []
    for c in range(NCORES):
        m = core1 == c
        idx1_all.append(src1_local[m])
        dl1_all.append(dst1[m])
        ew1_all.append(ew1[m])
    runs1 = [(0, _slot_blocks(list(range(T1)), SLOT1))]
    sel_runs1 = {}
    for c in range(NCORES):
        sel_runs1[(0, c)] = np.ones(len(idx1_all[c]), bool)
    prog1 = pk.pack_layer(1, runs1, idx1_all, dl1_all, ew1_all, sel_runs1)

    # ---- spans (gather calls) ----
    spans = []
    for layer, prog in ((0, prog0), (1, prog1)):
        for (table, a, b) in prog:
            g = a
            while g < b:
                e = min(g + CALL_G, b)
                spans.append((layer, table, g, e))
                g = e

    # ---- assemble per-core arrays ----
    nv = len(pk.drv)
    na = len(pk.drna)
    in_maps = []
    for c in range(NCORES):
        idxbuf = np.concatenate([blk[c] for blk in pk.idx_cols], axis=1) \
            if pk.idx_cols else np.zeros((128, 0), np.int16)
        drv = np.stack([col[c] for col in pk.drv], axis=1) if nv else \
            np.zeros((128, 0), np.float32)
        ewv = np.stack([col[c] for col in pk.ewv], axis=1) if nv else \
            np.zeros((128, 0), np.float32)
        drna = np.stack([col[c] for col in pk.drna], axis=1) if na else \
            np.zeros((128, 1), np.float32)
        ewa = np.stack([col[c] for col in pk.ewa], axis=1) if na else \
            np.zeros((128, 1), np.float32)
        ewna = np.stack([col[c] for col in pk.ewna], axis=1) if na else \
            np.zeros((128, 1), np.float32)
        in_maps.append(dict(x=np.asarray(x_bf), idxs=idxbuf, drv=drv, ewv=ewv,
                            drna=drna, ewa=ewa, ewna=ewna))
    return pk, spans, in_maps


def _build_program(pk, spans, nv, na, nidxcol):
    builds = pk.builds
    g_total = pk.g_total
    # group -> (span index, col in span)
    g_span = {}
    span_icol = []          # idx-col offset of each span
    off = 0
    for si, (layer, table, a, b) in enumerate(spans):
        span_icol.append(off)
        for g in range(a, b):
            g_span[g] = (si, g - a)
        off += (b - a) * 8

    # bank/quarter assignment + start/stop
    def bank_info(layer, t):
        if layer == 0:
            p = 0 if t < PASS0 else 1
            lt = t - (0 if p == 0 else PASS0)
            return (0, p, lt // 4), lt % 4
        else:
            return (1, t // 32, (t // 4) % 8), t % 4
    first_b = {}
    last_b = {}
    for i, (layer, g, t, eng, width) in enumerate(builds):
        for w in range(width):
            key, q = bank_info(layer, t + w)
            first_b.setdefault(key, (i, w))
            last_b[key] = (i, w)

    nc = bacc.Bacc("TRN2", target_bir_lowering=False, debug=False,
                   num_devices=NCORES)
    x_d = nc.dram_tensor("x", [N_SRC0, D], BF16, kind="ExternalInput")
    idxs_d = nc.dram_tensor("idxs", [128, nidxcol], I16, kind="ExternalInput")
    drv_d = nc.dram_tensor("drv", [128, max(nv, 1)], F32, kind="ExternalInput")
    ewv_d = nc.dram_tensor("ewv", [128, max(nv, 1)], F32, kind="ExternalInput")
    drna_d = nc.dram_tensor("drna", [128, max(na, 1)], F32,
                            kind="ExternalInput")
    ewa_d = nc.dram_tensor("ewa", [128, max(na, 1)], F32,
                           kind="ExternalInput")
    ewna_d = nc.dram_tensor("ewna", [128, max(na, 1)], F32,
                            kind="ExternalInput")
    h0_d = nc.dram_tensor("h0", [SLICE0, D], BF16)
    out_d = nc.dram_tensor("part", [T1 * 128, D], F32, kind="ExternalOutput")

    with tile.TileContext(nc) as tc:
        with (
            tc.tile_pool(name="const", bufs=1) as cpool,
            tc.tile_pool(name="mpool", bufs=6) as mpool,
            tc.tile_pool(name="spool", bufs=12) as spool,
            tc.tile_pool(name="sqpool", bufs=6) as sqpool,
            tc.tile_pool(name="stage", bufs=2) as stpool,
            tc.tile_pool(name="psum", bufs=1, space="PSUM") as ppool,
        ):
            iota32 = cpool.tile([128, 256], I32)
            iotabf = cpool.tile([128, 256], BF16)
            nc.gpsimd.iota(iota32[:], pattern=[[1, 256]], base=0,
                           channel_multiplier=0)
            nc.vector.tensor_copy(iotabf[:], iota32[:])

            idxs = cpool.tile([128, nidxcol], I16)
            drv = cpool.tile([128, max(nv, 1)], F32)
            ewv = cpool.tile([128, max(nv, 1)], F32)
            drna = cpool.tile([128, max(na, 1)], F32)
            ewa = cpool.tile([128, max(na, 1)], F32)
            ewna = cpool.tile([128, max(na, 1)], F32)
            # interleave loads so the first chunk of every array lands early;
            # the very first chunk of each array is small to unblock compute
            NLOAD = 6
            chunks = []
            for t_, d_ in ((idxs, idxs_d), (drv, drv_d), (ewv, ewv_d),
                           (drna, drna_d), (ewa, ewa_d), (ewna, ewna_d)):
                n = t_.shape[1]
                first = max(1, n // 24)
                step = -(-(n - first) // (NLOAD - 1))
                cl = [(t_, d_, 0, min(first, n))]
                for i in range(NLOAD - 1):
                    a = first + i * step
                    b = min(first + (i + 1) * step, n)
                    if a < b:
                        cl.append((t_, d_, a, b))
                chunks.append(cl)
            for i in range(NLOAD):
                for cl in chunks:
                    if i < len(cl):
                        t_, d_, a, b = cl[i]
                        nc.sync.dma_start(t_[:, a:b], d_[:, a:b])

            h0acc = cpool.tile([128, T0 * 128], BF16)

            banks = {}

            def get_bank(key):
                if key not in banks:
                    banks[key] = [ppool.tile([128, 512], F32,
                                             name=f"bk{key[2]}"),
                                  False]
                return banks[key][0]

            # walk builds in order; manage spans/gathers lazily
            mtiles = {}
            vi = 0
            ai = 0

            def ensure_span(si):
                if si in mtiles:
                    return mtiles[si]
                layer, table, a, b = spans[si]
                L = b - a
                mt = mpool.tile([128, CALL_G, 128], BF16, name="mt")
                tbl = x_d[table * CHUNK:(table + 1) * CHUNK, :] if layer == 0 \
                    else h0_d[:]
                ic = span_icol[si]
                nc.gpsimd.dma_gather(
                    mt[:, :L, :], tbl, idxs[:, ic:ic + L * 8],
                    num_idxs=L * 128, num_idxs_reg=L * 128, elem_size=128)
                mtiles.clear()
                mtiles[si] = mt
                return mt

            copy_after = {}     # build index -> list of copy ops
            # L0: whole-bank copies at end of each pass
            lastb_pass = {}
            for i, (layer, g, t, eng, width) in enumerate(builds):
                if layer == 0:
                    p = 0 if t < PASS0 else 1
                    lastb_pass[p] = i
            for p in (0, 1):
                ntile = PASS0 if p == 0 else T0 - PASS0
                nbank = -(-ntile // 4)
                ops = []
                for b in range(nbank):
                    ncols = min(4, ntile - b * 4) * 128
                    ops.append(('L0', p, b, ncols))
                ops.append(('H0', p))
                copy_after.setdefault(lastb_pass[p], []).extend(ops)
            # L1: full-bank copies, at the bank's last build
            lastb_t1 = {}
            for i, (layer, g, t, eng, width) in enumerate(builds):
                if layer == 1:
                    for w in range(width):
                        lastb_t1[t + w] = i
            for jb in range(T1 // 4):
                i = max(lastb_t1[4 * jb + k] for k in range(4))
                copy_after.setdefault(i, []).append(('L1', jb))

            stage_tiles = {}

            def do_copies(items):
                for op in items:
                    if op[0] == 'L0':
                        _, p, b, ncols = op
                        base = (0 if p == 0 else PASS0) * 128
                        bk = banks[(0, p, b)][0]
                        a = base + b * 512
                        nc.scalar.activation(
                            h0acc[:, a: a + ncols],
                            bk[:, :ncols], AF.Copy, bias=0.0, scale=1.0)
                        # store this bank's h0 rows immediately so the L1
                        # gather table completes as soon as possible
                        dram = h0_d[a:a + ncols, :].rearrange(
                            "(t p) d -> p t d", p=128)
                        nc.sync.dma_start(
                            dram, h0acc[:, a:a + ncols].rearrange(
                                "p (t d) -> p t d", d=128))
                    elif op[0] == 'H0':
                        pass
                    else:
                        _, jb = op
                        t0 = 4 * jb
                        key, q0 = bank_info(1, t0)
                        bk = banks[key][0]
                        jblk = t0 // STAGE_T
                        lt = t0 % STAGE_T
                        if jblk not in stage_tiles:
                            stage_tiles[jblk] = stpool.tile(
                                [128, STAGE_T * 128], F32, name="stg")
                        nc.scalar.activation(
                            stage_tiles[jblk][:, lt * 128:(lt + 4) * 128],
                            bk[:, :512],
                            AF.Copy, bias=0.0, scale=1.0)
                        last_blk = jblk == T1 // STAGE_T - 1
                        parts = ((12, 0, 16), (STAGE_T - 4, 16, STAGE_T)) \
                            if last_blk else ((STAGE_T - 4, 0, STAGE_T),)
                        for (trig, c0, c1) in parts:
                            if lt != trig:
                                continue
                            rows = STAGE_T * 128
                            dram = out_d[jblk * rows + c0 * 128:
                                         jblk * rows + c1 * 128, :] \
                                .rearrange("(t p) d -> p t d", p=128)
                            nc.sync.dma_start(
                                dram,
                                stage_tiles[jblk][:, c0 * 128:c1 * 128]
                                .rearrange("p (t d) -> p t d", d=128))
                            if c1 == STAGE_T:
                                del stage_tiles[jblk]

            for i, (layer, g, t, eng, width) in enumerate(builds):
                si, col = g_span[g]
                mt = ensure_span(si)
                nw = width * 128
                if eng == 'V':
                    S = spool.tile([128, 256], BF16, name="Sv")
                    nc.vector.tensor_scalar(
                        S[:, :nw], iotabf[:, :nw], drv[:, vi:vi + 1],
                        ewv[:, vi:vi + 1],
                        mybir.AluOpType.is_equal, mybir.AluOpType.mult)
                    vi += 1
                else:
                    sq = sqpool.tile([128, 256], BF16, name="sq")
                    nc.scalar.activation(sq[:, :nw], iotabf[:, :nw], AF.Square,
                                         bias=drna[:, ai:ai + 1], scale=1.0)
                    S = spool.tile([128, 256], BF16, name="Sa")
                    nc.scalar.activation(S[:, :nw], sq[:, :nw], AF.Relu,
                                         bias=ewa[:, ai:ai + 1],
                                         scale=ewna[:, ai:ai + 1])
                    ai += 1
                for w in range(width):
                    key, q = bank_info(layer, t + w)
                    bk = get_bank(key)
                    nc.tensor.matmul(bk[:, q * 128:(q + 1) * 128],
                                     S[:, w * 128:(w + 1) * 128],
                                     mt[:, col, :],
                                     start=(first_b[key] == (i, w)),
                                     stop=(last_b[key] == (i, w)))
                if i in copy_after:
                    do_copies(copy_after[i])

    nc.compile()
    return nc


def kernel(x, src0, dst0, ew0, src1, dst1, ew1, n_dst0, n_dst1):
    global _last_results, _last_nc
    t_start = time.time()
    x = np.asarray(x, dtype=np.float32)
    src0 = np.asarray(src0).astype(np.int64)
    dst0 = np.asarray(dst0).astype(np.int64)
    ew0 = np.asarray(ew0, dtype=np.float32)
    src1 = np.asarray(src1).astype(np.int64)
    dst1 = np.asarray(dst1).astype(np.int64)
    ew1 = np.asarray(ew1, dtype=np.float32)

    x_bf = x.astype(ml_dtypes.bfloat16)

    pk, spans, in_maps = _pack(x_bf, src0, dst0, ew0, src1, dst1, ew1)
    nv = len(pk.drv)
    na = len(pk.drna)
    nidxcol = in_maps[0]["idxs"].shape[1]
    t_pack = time.time()

    nc = _build_program(pk, spans, nv, na, nidxcol)
    _last_nc = nc
    t_build = time.time()

    trace = bool(int(os.environ.get("KBENCH_TRACE", "0")))
    try:
        res = run_bass_kernel_spmd(nc, in_maps, list(range(NCORES)),
                                   trace=trace)
    except ModuleNotFoundError:
        res = run_bass_kernel_spmd(nc, in_maps, list(range(NCORES)),
                                   trace=False)
    _last_results = res
    t_run = time.time()
    print(f"[kernel] pack {t_pack - t_start:.1f}s build+compile "
          f"{t_build - t_pack:.1f}s run {t_run - t_build:.1f}s "
          f"groups={pk.g_total} builds={len(pk.builds)} nv={nv} na={na}",
          file=sys.stderr)

    out = np.zeros((T1 * 128, D), np.float32)
    for c in range(NCORES):
        out += res.results[c]["part"]
    return out[: int(n_dst1)]


# revision 31
# speedup vs baseline: 1.5733x; 1.0063x over previous
"""LightGCN 2-layer propagation on 8 TRN2 NeuronCores.

Layer 0 (1.6M edges, x[100000,128] -> h0[50000,128]): dst-sharded. Core c owns
49 dst tiles of 128 rows. Edges are grouped into slots of paired dst tiles per
x-chunk (4 chunks of 25000 rows so gather indices fit int16), sorted by dst.
Edge source rows are gathered (bf16) via GPSIMD dma_gather; a one-hot matrix
S[e, d] = (iota == dst_rel[e]) * ew[e] is built per 128-edge group (on DVE via
tensor_scalar, a fraction on ACT via Square+Relu), and psum += S.T @ M
accumulates per dst tile. PSUM banks hold 4 dst tiles each (quarter regions,
one accumulation chain per bank), letting a tile integrate all 4 chunks in one
chain: L0 runs as 2 passes (28 + 21 tiles) over the 4 chunks. ACT copies psum
quarters into a bf16 h0 buffer, stored to DRAM with one rearranged DMA per
pass.
Layer 1 (800K edges, h0 -> out[25000,128]): src-sharded. Core c takes edges
whose src lies in its own h0 slice, gathers from its h0, accumulates over all
196 dst tiles (paired slots, 8-bank rotation), stages 28 tiles per f32 buffer
and stores 7 blocks; the host sums the 8 partial outputs.

SPMD: one program for all cores. Per-slot group counts are max'd across cores;
slack edges are padded (idx 0, dst sentinel -1, ew 0). Where a 128-edge group
straddles both tiles of a slot on any core, S is built per (group, tile) with
dr relative to that tile (out-of-tile edges never match iota, so S rows are 0).
"""
import os
import sys
import time

sys.path.insert(0, "/opt/trn_rl_repo")

import numpy as np
import ml_dtypes

import concourse.bacc as bacc
import concourse.mybir as mybir
from concourse import tile
from concourse.bass_utils import run_bass_kernel_spmd

BF16 = mybir.dt.bfloat16
F32 = mybir.dt.float32
I16 = mybir.dt.int16
I32 = mybir.dt.int32
AF = mybir.ActivationFunctionType

N_SRC0, N_DST0, N_DST1 = 100000, 50000, 25000
D = 128
NCORES = 8
T0 = 49            # dst tiles per core, layer 0
SLICE0 = T0 * 128  # 6272 dst rows per core
NCHUNK = 4
CHUNK = 25000
T1 = 196           # dst tiles, layer 1
CALL_G = 8         # gather-call size in 128-edge groups (1024 indices)
PASS0 = 28         # L0 pass-0 tiles (7 psum banks); pass 1 gets 21
ACTK0 = int(os.environ.get("KB_ACTK0", "9"))   # L0: every k-th S build on ACT
ACTK1 = int(os.environ.get("KB_ACTK1", "14"))  # L1: every k-th S build on ACT
STAGE_T = 28       # L1 out tiles per staging buffer (196 = 7*28)

_last_results = None
_last_nc = None


SLOT0 = int(os.environ.get("KB_SLOT0", "14"))  # L0 slot size (tiles)
SLOT1 = int(os.environ.get("KB_SLOT1", "16"))  # L1 slot size (tiles)


def _slot_blocks(tiles, size):
    out = []
    i = 0
    while i < len(tiles):
        out.append(tuple(tiles[i : i + size]))
        i += size
    return out


class _Packer:
    """Accumulates the SPMD program structure + per-core data streams."""

    def __init__(self):
        self.idx_cols = []      # per core: list of [128, L*8] int16 blocks
        self.builds = []        # program: (layer, g_global, tile, engine)
        self.drv = []           # per core: list of len-128 f32 cols (DVE)
        self.ewv = []
        self.drna = []          # per core: ACT cols (-dr, ew, -ew)
        self.ewa = []
        self.ewna = []
        self.spans = []         # program: (layer, table, gstart, gend)
        self.g_total = 0
        self.build_no = 0

    def pack_layer(self, layer, runs, idx_all, dst_local_all, ew_all, sel_runs):
        """runs: list of (table_id, [slot tuples of tile ids]).
        sel_runs[(r, c)] -> bool mask of core c's edges for run r.
        idx_all/dst_local_all/ew_all: per-core arrays aligned with the masks.
        Returns program info: list of per-run group ranges + build entries.
        """
        prog = []
        for ri, (table, slots) in enumerate(runs):
            run_g0 = self.g_total
            for slot in slots:
                per_core = []
                for c in range(NCORES):
                    m = sel_runs[(ri, c)]
                    dl = dst_local_all[c][m]
                    tsel = dl // 128
                    smask = np.isin(tsel, slot)
                    order = np.argsort(dl[smask], kind="stable")
                    per_core.append((idx_all[c][m][smask][order],
                                     dl[smask][order],
                                     ew_all[c][m][smask][order]))
                n = np.array([len(p[0]) for p in per_core])
                gs = max(1, -(-int(n.max()) // 128))
                # cumulative edge counts per tile boundary (slot tiles are
                # contiguous and each core's edges are dst-sorted)
                m_ = len(slot)
                cums = np.zeros((NCORES, m_), np.int64)
                for c in range(NCORES):
                    dl = per_core[c][1]
                    for i_t, t in enumerate(slot):
                        cums[c, i_t] = np.searchsorted(dl, (t + 1) * 128)
                # pad each core to gs*128
                for c in range(NCORES):
                    pad = gs * 128 - n[c]
                    ii = np.concatenate([per_core[c][0],
                                         np.zeros(pad, np.int64)])
                    dd = np.concatenate([per_core[c][1],
                                         np.full(pad, -1, np.int64)])
                    ee = np.concatenate([per_core[c][2],
                                         np.zeros(pad, np.float32)])
                    per_core[c] = (ii, dd, ee)
                # per-tile build window [lo, hi) over groups; every tile gets
                # >= 1 build so its psum quarter is written and copied
                los = []
                his = []
                for i_t in range(m_):
                    lo = 0 if i_t == 0 else int((cums[:, i_t - 1] // 128)
                                                .min())
                    hi = -(-int(cums[:, i_t].max()) // 128)
                    lo = min(lo, gs - 1)
                    hi = min(max(hi, lo + 1), gs)
                    los.append(lo)
                    his.append(hi)
                sched = [[] for _ in range(gs)]
                for i_t, t in enumerate(slot):
                    for g in range(los[i_t], his[i_t]):
                        sched[g].append(t)
                # emit; adjacent-tile builds in a group merge into ONE wide
                # [128,256] S build (dr relative to the lower tile)
                for g in range(gs):
                    gg = self.g_total + g
                    tl = sched[g]
                    items = []
                    j = 0
                    while j < len(tl):
                        if j + 1 < len(tl) and tl[j + 1] == tl[j] + 1:
                            items.append((g, tl[j], 2))
                            j += 2
                        else:
                            items.append((g, tl[j], 1))
                            j += 1
                    for (g_, t, width) in items:
                        actk = ACTK0 if layer == 0 else ACTK1
                        eng = 'A' if (actk > 0 and
                                      self.build_no % actk == actk - 1) \
                            else 'V'
                        self.build_no += 1
                        for c in range(NCORES):
                            dd = per_core[c][1][g * 128:(g + 1) * 128]
                            ee = per_core[c][2][g * 128:(g + 1) * 128]
                            rel = (dd - t * 128).astype(np.float32)
                            if eng == 'V':
                                if c == 0:
                                    self.drv.append([])
                                    self.ewv.append([])
                                self.drv[-1].append(rel)
                                self.ewv[-1].append(ee.astype(np.float32))
                            else:
                                if c == 0:
                                    self.drna.append([])
                                    self.ewa.append([])
                                    self.ewna.append([])
                                self.drna[-1].append(-rel)
                                self.ewa[-1].append(ee.astype(np.float32))
                                self.ewna[-1].append(-ee.astype(np.float32))
                        self.builds.append((layer, gg, t, eng, width))
                    # idx stream for this group, per core
                    for c in range(NCORES):
                        ii = per_core[c][0][g * 128:(g + 1) * 128]
                        if c == 0:
                            self.idx_cols.append([])
                        w = np.ascontiguousarray(
                            ii.astype(np.int16).reshape(-1, 16).T)
                        self.idx_cols[-1].append(np.tile(w, (8, 1)))
                self.g_total += gs
            prog.append((table, run_g0, self.g_total))
        return prog


def _pack(x_bf, src0, dst0, ew0, src1, dst1, ew1):
    pk = _Packer()

    # ---- balanced L0 tile->core assignment ----
    # Global dst tiles are assigned to (core, local slot) so that the 8 tiles
    # sharing a slot have similar edge counts: SPMD group counts are maxima
    # across cores, so similar counts minimize padding.
    NGT = T0 * NCORES                    # 392 slots; tile 391 is empty pad
    gt0 = dst0 // 128
    cnt_t = np.bincount(gt0, minlength=NGT)
    order = np.argsort(-cnt_t, kind="stable")
    singles = order[-NCORES:]
    rest = order[:-NCORES]
    pairs = rest.reshape(-1, 2)
    porder = pairs[np.argsort(-cnt_t[pairs].sum(1), kind="stable")]
    core_of = np.zeros(NGT, np.int64)
    local_of = np.zeros(NGT, np.int64)
    nslot = len(porder) // NCORES        # 24 pair slots per core
    for s in range(nslot):
        for c in range(NCORES):
            a, b = porder[NCORES * s + c]
            core_of[a] = c
            local_of[a] = 2 * s
            core_of[b] = c
            local_of[b] = 2 * s + 1
    for c in range(NCORES):
        core_of[singles[c]] = c
        local_of[singles[c]] = T0 - 1

    # ---- layer 0 selection ----
    core0 = core_of[gt0]
    chunk0 = src0 // CHUNK
    dst_local0 = local_of[gt0] * 128 + dst0 % 128
    pass_tiles = [list(range(0, PASS0)), list(range(PASS0, T0))]
    runs0 = []
    sel0 = {}
    idx0_all, dl0_all, ew0_all = [], [], []
    for c in range(NCORES):
        m = core0 == c
        idx0_all.append((src0[m] % CHUNK))
        dl0_all.append(dst_local0[m])
        ew0_all.append(ew0[m])
        sel0_chunk = chunk0[m]
        sel0[c] = sel0_chunk
    ri = 0
    sel_runs0 = {}
    for p in range(2):
        tset = set(pass_tiles[p])
        for k in range(NCHUNK):
            slots = _slot_blocks(pass_tiles[p], SLOT0)
            runs0.append((k, slots))
            for c in range(NCORES):
                tl = dl0_all[c] // 128
                sel_runs0[(ri, c)] = (sel0[c] == k) & np.isin(
                    tl, pass_tiles[p])
            ri += 1
    prog0 = pk.pack_layer(0, runs0, idx0_all, dl0_all, ew0_all, sel_runs0)

    # ---- layer 1 selection (src rows follow the L0 tile permutation) ----
    gt1 = src1 // 128
    core1 = core_of[gt1]
    src1_local = local_of[gt1] * 128 + src1 % 128
    idx1_all, dl1_all, ew1_all = [], [], []
    for c in range(NCORES):
        m = core1 == c
        idx1_all.append(src1_local[m])
        dl1_all.append(dst1[m])
        ew1_all.append(ew1[m])
    runs1 = [(0, _slot_blocks(list(range(T1)), SLOT1))]
    sel_runs1 = {}
    for c in range(NCORES):
        sel_runs1[(0, c)] = np.ones(len(idx1_all[c]), bool)
    prog1 = pk.pack_layer(1, runs1, idx1_all, dl1_all, ew1_all, sel_runs1)

    # ---- spans (gather calls) ----
    spans = []
    for layer, prog in ((0, prog0), (1, prog1)):
        for (table, a, b) in prog:
            g = a
            while g < b:
                e = min(g + CALL_G, b)
                spans.append((layer, table, g, e))
                g = e

    # ---- assemble per-core arrays ----
    nv = len(pk.drv)
    na = len(pk.drna)
    in_maps = []
    for c in range(NCORES):
        idxbuf = np.concatenate([blk[c] for blk in pk.idx_cols], axis=1) \
            if pk.idx_cols else np.zeros((128, 0), np.int16)
        drv = np.stack([col[c] for col in pk.drv], axis=1) if nv else \
            np.zeros((128, 0), np.float32)
        ewv = np.stack([col[c] for col in pk.ewv], axis=1) if nv else \
            np.zeros((128, 0), np.float32)
        drna = np.stack([col[c] for col in pk.drna], axis=1) if na else \
            np.zeros((128, 1), np.float32)
        ewa = np.stack([col[c] for col in pk.ewa], axis=1) if na else \
            np.zeros((128, 1), np.float32)
        ewna = np.stack([col[c] for col in pk.ewna], axis=1) if na else \
            np.zeros((128, 1), np.float32)
        in_maps.append(dict(x=np.asarray(x_bf), idxs=idxbuf, drv=drv, ewv=ewv,
                            drna=drna, ewa=ewa, ewna=ewna))
    return pk, spans, in_maps


def _build_program(pk, spans, nv, na, nidxcol):
    builds = pk.builds
    g_total = pk.g_total
    # group -> (span index, col in span)
    g_span = {}
    span_icol = []          # idx-col offset of each span
    off = 0
    for si, (layer, table, a, b) in enumerate(spans):
        span_icol.append(off)
        for g in range(a, b):
            g_span[g] = (si, g - a)
        off += (b - a) * 8

    # bank/quarter assignment + start/stop
    def bank_info(layer, t):
        if layer == 0:
            p = 0 if t < PASS0 else 1
            lt = t - (0 if p == 0 else PASS0)
            return (0, p, lt // 4), lt % 4
        else:
            return (1, t // 32, (t // 4) % 8), t % 4
    first_b = {}
    last_b = {}
    for i, (layer, g, t, eng, width) in enumerate(builds):
        for w in range(width):
            key, q = bank_info(layer, t + w)
            first_b.setdefault(key, (i, w))
            last_b[key] = (i, w)

    nc = bacc.Bacc("TRN2", target_bir_lowering=False, debug=False,
                   num_devices=NCORES)
    x_d = nc.dram_tensor("x", [N_SRC0, D], BF16, kind="ExternalInput")
    idxs_d = nc.dram_tensor("idxs", [128, nidxcol], I16, kind="ExternalInput")
    drv_d = nc.dram_tensor("drv", [128, max(nv, 1)], F32, kind="ExternalInput")
    ewv_d = nc.dram_tensor("ewv", [128, max(nv, 1)], F32, kind="ExternalInput")
    drna_d = nc.dram_tensor("drna", [128, max(na, 1)], F32,
                            kind="ExternalInput")
    ewa_d = nc.dram_tensor("ewa", [128, max(na, 1)], F32,
                           kind="ExternalInput")
    ewna_d = nc.dram_tensor("ewna", [128, max(na, 1)], F32,
                            kind="ExternalInput")
    h0_d = nc.dram_tensor("h0", [SLICE0, D], BF16)
    out_d = nc.dram_tensor("part", [T1 * 128, D], F32, kind="ExternalOutput")

    with tile.TileContext(nc) as tc:
        with (
            tc.tile_pool(name="const", bufs=1) as cpool,
            tc.tile_pool(name="mpool", bufs=6) as mpool,
            tc.tile_pool(name="spool", bufs=12) as spool,
            tc.tile_pool(name="sqpool", bufs=6) as sqpool,
            tc.tile_pool(name="stage", bufs=2) as stpool,
            tc.tile_pool(name="psum", bufs=1, space="PSUM") as ppool,
        ):
            iota32 = cpool.tile([128, 256], I32)
            iotabf = cpool.tile([128, 256], BF16)
            nc.gpsimd.iota(iota32[:], pattern=[[1, 256]], base=0,
                           channel_multiplier=0)
            nc.vector.tensor_copy(iotabf[:], iota32[:])

            idxs = cpool.tile([128, nidxcol], I16)
            drv = cpool.tile([128, max(nv, 1)], F32)
            ewv = cpool.tile([128, max(nv, 1)], F32)
            drna = cpool.tile([128, max(na, 1)], F32)
            ewa = cpool.tile([128, max(na, 1)], F32)
            ewna = cpool.tile([128, max(na, 1)], F32)
            # interleave loads so the first chunk of every array lands early;
            # the very first chunk of each array is small to unblock compute
            NLOAD = 6
            chunks = []
            for t_, d_ in ((idxs, idxs_d), (drv, drv_d), (ewv, ewv_d),
                           (drna, drna_d), (ewa, ewa_d), (ewna, ewna_d)):
                n = t_.shape[1]
                first = max(1, n // 24)
                step = -(-(n - first) // (NLOAD - 1))
                cl = [(t_, d_, 0, min(first, n))]
                for i in range(NLOAD - 1):
                    a = first + i * step
                    b = min(first + (i + 1) * step, n)
                    if a < b:
                        cl.append((t_, d_, a, b))
                chunks.append(cl)
            for i in range(NLOAD):
                for cl in chunks:
                    if i < len(cl):
                        t_, d_, a, b = cl[i]
                        nc.sync.dma_start(t_[:, a:b], d_[:, a:b])

            h0acc = cpool.tile([128, T0 * 128], BF16)

            banks = {}

            def get_bank(key):
                if key not in banks:
                    banks[key] = [ppool.tile([128, 512], F32,
                                             name=f"bk{key[2]}"),
                                  False]
                return banks[key][0]

            # walk builds in order; manage spans/gathers lazily
            mtiles = {}
            vi = 0
            ai = 0

            def ensure_span(si):
                if si in mtiles:
                    return mtiles[si]
                layer, table, a, b = spans[si]
                L = b - a
                mt = mpool.tile([128, CALL_G, 128], BF16, name="mt")
                tbl = x_d[table * CHUNK:(table + 1) * CHUNK, :] if layer == 0 \
                    else h0_d[:]
                ic = span_icol[si]
                nc.gpsimd.dma_gather(
                    mt[:, :L, :], tbl, idxs[:, ic:ic + L * 8],
                    num_idxs=L * 128, num_idxs_reg=L * 128, elem_size=128)
                mtiles.clear()
                mtiles[si] = mt
                return mt

            copy_after = {}     # build index -> list of copy ops
            # L0: whole-bank copies at end of each pass
            lastb_pass = {}
            for i, (layer, g, t, eng, width) in enumerate(builds):
                if layer == 0:
                    p = 0 if t < PASS0 else 1
                    lastb_pass[p] = i
            for p in (0, 1):
                ntile = PASS0 if p == 0 else T0 - PASS0
                nbank = -(-ntile // 4)
                ops = []
                for b in range(nbank):
                    ncols = min(4, ntile - b * 4) * 128
                    ops.append(('L0', p, b, ncols))
                ops.append(('H0', p))
                copy_after.setdefault(lastb_pass[p], []).extend(ops)
            # L1: full-bank copies, at the bank's last build
            lastb_t1 = {}
            for i, (layer, g, t, eng, width) in enumerate(builds):
                if layer == 1:
                    for w in range(width):
                        lastb_t1[t + w] = i
            for jb in range(T1 // 4):
                i = max(lastb_t1[4 * jb + k] for k in range(4))
                copy_after.setdefault(i, []).append(('L1', jb))

            stage_tiles = {}

            def do_copies(items):
                for op in items:
                    if op[0] == 'L0':
                        _, p, b, ncols = op
                        base = (0 if p == 0 else PASS0) * 128
                        bk = banks[(0, p, b)][0]
                        a = base + b * 512
                        nc.scalar.activation(
                            h0acc[:, a: a + ncols],
                            bk[:, :ncols], AF.Copy, bias=0.0, scale=1.0)
                        # store this bank's h0 rows immediately so the L1
                        # gather table completes as soon as possible
                        dram = h0_d[a:a + ncols, :].rearrange(
                            "(t p) d -> p t d", p=128)
                        nc.sync.dma_start(
                            dram, h0acc[:, a:a + ncols].rearrange(
                                "p (t d) -> p t d", d=128))
                    elif op[0] == 'H0':
                        pass
                    else:
                        _, jb = op
                        t0 = 4 * jb
                        key, q0 = bank_info(1, t0)
                        bk = banks[key][0]
                        jblk = t0 // STAGE_T
                        lt = t0 % STAGE_T
                        if jblk not in stage_tiles:
                            stage_tiles[jblk] = stpool.tile(
                                [128, STAGE_T * 128], F32, name="stg")
                        nc.scalar.activation(
                            stage_tiles[jblk][:, lt * 128:(lt + 4) * 128],
                            bk[:, :512],
                            AF.Copy, bias=0.0, scale=1.0)
                        last_blk = jblk == T1 // STAGE_T - 1
                        parts = ((12, 0, 16), (20, 16, 24),
                                 (STAGE_T - 4, 24, STAGE_T)) \
                            if last_blk else ((STAGE_T - 4, 0, STAGE_T),)
                        for (trig, c0, c1) in parts:
                            if lt != trig:
                                continue
                            rows = STAGE_T * 128
                            dram = out_d[jblk * rows + c0 * 128:
                                         jblk * rows + c1 * 128, :] \
                                .rearrange("(t p) d -> p t d", p=128)
                            nc.sync.dma_start(
                                dram,
                                stage_tiles[jblk][:, c0 * 128:c1 * 128]
                                .rearrange("p (t d) -> p t d", d=128))
                            if c1 == STAGE_T:
                                del stage_tiles[jblk]

            for i, (layer, g, t, eng, width) in enumerate(builds):
                si, col = g_span[g]
                mt = ensure_span(si)
                nw = width * 128
                if eng == 'V':
                    S = spool.tile([128, 256], BF16, name="Sv")
                    nc.vector.tensor_scalar(
                        S[:, :nw], iotabf[:, :nw], drv[:, vi:vi + 1],
                        ewv[:, vi:vi + 1],
                        mybir.AluOpType.is_equal, mybir.AluOpType.mult)
                    vi += 1
                else:
                    sq = sqpool.tile([128, 256], BF16, name="sq")
                    nc.scalar.activation(sq[:, :nw], iotabf[:, :nw], AF.Square,
                                         bias=drna[:, ai:ai + 1], scale=1.0)
                    S = spool.tile([128, 256], BF16, name="Sa")
                    nc.scalar.activation(S[:, :nw], sq[:, :nw], AF.Relu,
                                         bias=ewa[:, ai:ai + 1],
                                         scale=ewna[:, ai:ai + 1])
                    ai += 1
                for w in range(width):
                    key, q = bank_info(layer, t + w)
                    bk = get_bank(key)
                    nc.tensor.matmul(bk[:, q * 128:(q + 1) * 128],
                                     S[:, w * 128:(w + 1) * 128],
                                     mt[:, col, :],
                                     start=(first_b[key] == (i, w)),
                                     stop=(last_b[key] == (i, w)))
                if i in copy_after:
                    do_copies(copy_after[i])

    nc.compile()
    return nc


def kernel(x, src0, dst0, ew0, src1, dst1, ew1, n_dst0, n_dst1):
    global _last_results, _last_nc
    t_start = time.time()
    x = np.asarray(x, dtype=np.float32)
    src0 = np.asarray(src0).astype(np.int64)
    dst0 = np.asarray(dst0).astype(np.int64)
    ew0 = np.asarray(ew0, dtype=np.float32)
    src1 = np.asarray(src1).astype(np.int64)
    dst1 = np.asarray(dst1).astype(np.int64)
    ew1 = np.asarray(ew1, dtype=np.float32)

    x_bf = x.astype(ml_dtypes.bfloat16)

    pk, spans, in_maps = _pack(x_bf, src0, dst0, ew0, src1, dst1, ew1)
    nv = len(pk.drv)
    na = len(pk.drna)
    nidxcol = in_maps[0]["idxs"].shape[1]
    t_pack = time.time()

    nc = _build_program(pk, spans, nv, na, nidxcol)
    _last_nc = nc
    t_build = time.time()

    trace = bool(int(os.environ.get("KBENCH_TRACE", "0")))
    try:
        res = run_bass_kernel_spmd(nc, in_maps, list(range(NCORES)),
                                   trace=trace)
    except ModuleNotFoundError:
        res = run_bass_kernel_spmd(nc, in_maps, list(range(NCORES)),
                                   trace=False)
    _last_results = res
    t_run = time.time()
    print(f"[kernel] pack {t_pack - t_start:.1f}s build+compile "
          f"{t_build - t_pack:.1f}s run {t_run - t_build:.1f}s "
          f"groups={pk.g_total} builds={len(pk.builds)} nv={nv} na={na}",
          file=sys.stderr)

    out = np.zeros((T1 * 128, D), np.float32)
    for c in range(NCORES):
        out += res.results[c]["part"]
    return out[: int(n_dst1)]


# revision 32
# speedup vs baseline: 1.5784x; 1.0032x over previous
"""LightGCN 2-layer propagation on 8 TRN2 NeuronCores.

Layer 0 (1.6M edges, x[100000,128] -> h0[50000,128]): dst-sharded. Core c owns
49 dst tiles of 128 rows. Edges are grouped into slots of paired dst tiles per
x-chunk (4 chunks of 25000 rows so gather indices fit int16), sorted by dst.
Edge source rows are gathered (bf16) via GPSIMD dma_gather; a one-hot matrix
S[e, d] = (iota == dst_rel[e]) * ew[e] is built per 128-edge group (on DVE via
tensor_scalar, a fraction on ACT via Square+Relu), and psum += S.T @ M
accumulates per dst tile. PSUM banks hold 4 dst tiles each (quarter regions,
one accumulation chain per bank), letting a tile integrate all 4 chunks in one
chain: L0 runs as 2 passes (28 + 21 tiles) over the 4 chunks. ACT copies psum
quarters into a bf16 h0 buffer, stored to DRAM with one rearranged DMA per
pass.
Layer 1 (800K edges, h0 -> out[25000,128]): src-sharded. Core c takes edges
whose src lies in its own h0 slice, gathers from its h0, accumulates over all
196 dst tiles (paired slots, 8-bank rotation), stages 28 tiles per f32 buffer
and stores 7 blocks; the host sums the 8 partial outputs.

SPMD: one program for all cores. Per-slot group counts are max'd across cores;
slack edges are padded (idx 0, dst sentinel -1, ew 0). Where a 128-edge group
straddles both tiles of a slot on any core, S is built per (group, tile) with
dr relative to that tile (out-of-tile edges never match iota, so S rows are 0).
"""
import os
import sys
import time

sys.path.insert(0, "/opt/trn_rl_repo")

import numpy as np
import ml_dtypes

import concourse.bacc as bacc
import concourse.mybir as mybir
from concourse import tile
from concourse.bass_utils import run_bass_kernel_spmd

BF16 = mybir.dt.bfloat16
F32 = mybir.dt.float32
I16 = mybir.dt.int16
I32 = mybir.dt.int32
AF = mybir.ActivationFunctionType

N_SRC0, N_DST0, N_DST1 = 100000, 50000, 25000
D = 128
NCORES = 8
T0 = 49            # dst tiles per core, layer 0
SLICE0 = T0 * 128  # 6272 dst rows per core
NCHUNK = 4
CHUNK = 25000
T1 = 196           # dst tiles, layer 1
CALL_G = 8         # gather-call size in 128-edge groups (1024 indices)
PASS0 = 28         # L0 pass-0 tiles (7 psum banks); pass 1 gets 21
ACTK0 = int(os.environ.get("KB_ACTK0", "9"))   # L0: every k-th S build on ACT
ACTK1 = int(os.environ.get("KB_ACTK1", "14"))  # L1: every k-th S build on ACT
STAGE_T = 28       # L1 out tiles per staging buffer (196 = 7*28)

_last_results = None
_last_nc = None


SLOT0 = int(os.environ.get("KB_SLOT0", "28"))  # L0 slot size (tiles)
SLOT1 = int(os.environ.get("KB_SLOT1", "16"))  # L1 slot size (tiles)


def _slot_blocks(tiles, size):
    out = []
    i = 0
    while i < len(tiles):
        out.append(tuple(tiles[i : i + size]))
        i += size
    return out


class _Packer:
    """Accumulates the SPMD program structure + per-core data streams."""

    def __init__(self):
        self.idx_cols = []      # per core: list of [128, L*8] int16 blocks
        self.builds = []        # program: (layer, g_global, tile, engine)
        self.drv = []           # per core: list of len-128 f32 cols (DVE)
        self.ewv = []
        self.drna = []          # per core: ACT cols (-dr, ew, -ew)
        self.ewa = []
        self.ewna = []
        self.spans = []         # program: (layer, table, gstart, gend)
        self.g_total = 0
        self.build_no = 0

    def pack_layer(self, layer, runs, idx_all, dst_local_all, ew_all, sel_runs):
        """runs: list of (table_id, [slot tuples of tile ids]).
        sel_runs[(r, c)] -> bool mask of core c's edges for run r.
        idx_all/dst_local_all/ew_all: per-core arrays aligned with the masks.
        Returns program info: list of per-run group ranges + build entries.
        """
        prog = []
        for ri, (table, slots) in enumerate(runs):
            run_g0 = self.g_total
            for slot in slots:
                per_core = []
                for c in range(NCORES):
                    m = sel_runs[(ri, c)]
                    dl = dst_local_all[c][m]
                    tsel = dl // 128
                    smask = np.isin(tsel, slot)
                    order = np.argsort(dl[smask], kind="stable")
                    per_core.append((idx_all[c][m][smask][order],
                                     dl[smask][order],
                                     ew_all[c][m][smask][order]))
                n = np.array([len(p[0]) for p in per_core])
                gs = max(1, -(-int(n.max()) // 128))
                # cumulative edge counts per tile boundary (slot tiles are
                # contiguous and each core's edges are dst-sorted)
                m_ = len(slot)
                cums = np.zeros((NCORES, m_), np.int64)
                for c in range(NCORES):
                    dl = per_core[c][1]
                    for i_t, t in enumerate(slot):
                        cums[c, i_t] = np.searchsorted(dl, (t + 1) * 128)
                # pad each core to gs*128
                for c in range(NCORES):
                    pad = gs * 128 - n[c]
                    ii = np.concatenate([per_core[c][0],
                                         np.zeros(pad, np.int64)])
                    dd = np.concatenate([per_core[c][1],
                                         np.full(pad, -1, np.int64)])
                    ee = np.concatenate([per_core[c][2],
                                         np.zeros(pad, np.float32)])
                    per_core[c] = (ii, dd, ee)
                # per-tile build window [lo, hi) over groups; every tile gets
                # >= 1 build so its psum quarter is written and copied
                los = []
                his = []
                for i_t in range(m_):
                    lo = 0 if i_t == 0 else int((cums[:, i_t - 1] // 128)
                                                .min())
                    hi = -(-int(cums[:, i_t].max()) // 128)
                    lo = min(lo, gs - 1)
                    hi = min(max(hi, lo + 1), gs)
                    los.append(lo)
                    his.append(hi)
                sched = [[] for _ in range(gs)]
                for i_t, t in enumerate(slot):
                    for g in range(los[i_t], his[i_t]):
                        sched[g].append(t)
                # emit; adjacent-tile builds in a group merge into ONE wide
                # [128,256] S build (dr relative to the lower tile)
                for g in range(gs):
                    gg = self.g_total + g
                    tl = sched[g]
                    items = []
                    j = 0
                    while j < len(tl):
                        if j + 1 < len(tl) and tl[j + 1] == tl[j] + 1:
                            items.append((g, tl[j], 2))
                            j += 2
                        else:
                            items.append((g, tl[j], 1))
                            j += 1
                    for (g_, t, width) in items:
                        actk = ACTK0 if layer == 0 else ACTK1
                        eng = 'A' if (actk > 0 and
                                      self.build_no % actk == actk - 1) \
                            else 'V'
                        self.build_no += 1
                        for c in range(NCORES):
                            dd = per_core[c][1][g * 128:(g + 1) * 128]
                            ee = per_core[c][2][g * 128:(g + 1) * 128]
                            rel = (dd - t * 128).astype(np.float32)
                            if eng == 'V':
                                if c == 0:
                                    self.drv.append([])
                                    self.ewv.append([])
                                self.drv[-1].append(rel)
                                self.ewv[-1].append(ee.astype(np.float32))
                            else:
                                if c == 0:
                                    self.drna.append([])
                                    self.ewa.append([])
                                    self.ewna.append([])
                                self.drna[-1].append(-rel)
                                self.ewa[-1].append(ee.astype(np.float32))
                                self.ewna[-1].append(-ee.astype(np.float32))
                        self.builds.append((layer, gg, t, eng, width))
                    # idx stream for this group, per core
                    for c in range(NCORES):
                        ii = per_core[c][0][g * 128:(g + 1) * 128]
                        if c == 0:
                            self.idx_cols.append([])
                        w = np.ascontiguousarray(
                            ii.astype(np.int16).reshape(-1, 16).T)
                        self.idx_cols[-1].append(np.tile(w, (8, 1)))
                self.g_total += gs
            prog.append((table, run_g0, self.g_total))
        return prog


def _pack(x_bf, src0, dst0, ew0, src1, dst1, ew1):
    pk = _Packer()

    # ---- balanced L0 tile->core assignment ----
    # Global dst tiles are assigned to (core, local slot) so that the 8 tiles
    # sharing a slot have similar edge counts: SPMD group counts are maxima
    # across cores, so similar counts minimize padding.
    NGT = T0 * NCORES                    # 392 slots; tile 391 is empty pad
    gt0 = dst0 // 128
    cnt_t = np.bincount(gt0, minlength=NGT)
    order = np.argsort(-cnt_t, kind="stable")
    singles = order[-NCORES:]
    rest = order[:-NCORES]
    pairs = rest.reshape(-1, 2)
    porder = pairs[np.argsort(-cnt_t[pairs].sum(1), kind="stable")]
    core_of = np.zeros(NGT, np.int64)
    local_of = np.zeros(NGT, np.int64)
    nslot = len(porder) // NCORES        # 24 pair slots per core
    for s in range(nslot):
        for c in range(NCORES):
            a, b = porder[NCORES * s + c]
            core_of[a] = c
            local_of[a] = 2 * s
            core_of[b] = c
            local_of[b] = 2 * s + 1
    for c in range(NCORES):
        core_of[singles[c]] = c
        local_of[singles[c]] = T0 - 1

    # ---- layer 0 selection ----
    core0 = core_of[gt0]
    chunk0 = src0 // CHUNK
    dst_local0 = local_of[gt0] * 128 + dst0 % 128
    pass_tiles = [list(range(0, PASS0)), list(range(PASS0, T0))]
    runs0 = []
    sel0 = {}
    idx0_all, dl0_all, ew0_all = [], [], []
    for c in range(NCORES):
        m = core0 == c
        idx0_all.append((src0[m] % CHUNK))
        dl0_all.append(dst_local0[m])
        ew0_all.append(ew0[m])
        sel0_chunk = chunk0[m]
        sel0[c] = sel0_chunk
    ri = 0
    sel_runs0 = {}
    for p in range(2):
        tset = set(pass_tiles[p])
        for k in range(NCHUNK):
            slots = _slot_blocks(pass_tiles[p], SLOT0)
            runs0.append((k, slots))
            for c in range(NCORES):
                tl = dl0_all[c] // 128
                sel_runs0[(ri, c)] = (sel0[c] == k) & np.isin(
                    tl, pass_tiles[p])
            ri += 1
    prog0 = pk.pack_layer(0, runs0, idx0_all, dl0_all, ew0_all, sel_runs0)

    # ---- layer 1 selection (src rows follow the L0 tile permutation) ----
    gt1 = src1 // 128
    core1 = core_of[gt1]
    src1_local = local_of[gt1] * 128 + src1 % 128
    idx1_all, dl1_all, ew1_all = [], [], []
    for c in range(NCORES):
        m = core1 == c
        idx1_all.append(src1_local[m])
        dl1_all.append(dst1[m])
        ew1_all.append(ew1[m])
    runs1 = [(0, _slot_blocks(list(range(T1)), SLOT1))]
    sel_runs1 = {}
    for c in range(NCORES):
        sel_runs1[(0, c)] = np.ones(len(idx1_all[c]), bool)
    prog1 = pk.pack_layer(1, runs1, idx1_all, dl1_all, ew1_all, sel_runs1)

    # ---- spans (gather calls) ----
    spans = []
    for layer, prog in ((0, prog0), (1, prog1)):
        for (table, a, b) in prog:
            g = a
            while g < b:
                e = min(g + CALL_G, b)
                spans.append((layer, table, g, e))
                g = e

    # ---- assemble per-core arrays ----
    nv = len(pk.drv)
    na = len(pk.drna)
    in_maps = []
    for c in range(NCORES):
        idxbuf = np.concatenate([blk[c] for blk in pk.idx_cols], axis=1) \
            if pk.idx_cols else np.zeros((128, 0), np.int16)
        drv = np.stack([col[c] for col in pk.drv], axis=1) if nv else \
            np.zeros((128, 0), np.float32)
        ewv = np.stack([col[c] for col in pk.ewv], axis=1) if nv else \
            np.zeros((128, 0), np.float32)
        drna = np.stack([col[c] for col in pk.drna], axis=1) if na else \
            np.zeros((128, 1), np.float32)
        ewa = np.stack([col[c] for col in pk.ewa], axis=1) if na else \
            np.zeros((128, 1), np.float32)
        ewna = np.stack([col[c] for col in pk.ewna], axis=1) if na else \
            np.zeros((128, 1), np.float32)
        in_maps.append(dict(x=np.asarray(x_bf), idxs=idxbuf, drv=drv, ewv=ewv,
                            drna=drna, ewa=ewa, ewna=ewna))
    return pk, spans, in_maps


def _build_program(pk, spans, nv, na, nidxcol):
    builds = pk.builds
    g_total = pk.g_total
    # group -> (span index, col in span)
    g_span = {}
    span_icol = []          # idx-col offset of each span
    off = 0
    for si, (layer, table, a, b) in enumerate(spans):
        span_icol.append(off)
        for g in range(a, b):
            g_span[g] = (si, g - a)
        off += (b - a) * 8

    # bank/quarter assignment + start/stop
    def bank_info(layer, t):
        if layer == 0:
            p = 0 if t < PASS0 else 1
            lt = t - (0 if p == 0 else PASS0)
            return (0, p, lt // 4), lt % 4
        else:
            return (1, t // 32, (t // 4) % 8), t % 4
    first_b = {}
    last_b = {}
    for i, (layer, g, t, eng, width) in enumerate(builds):
        for w in range(width):
            key, q = bank_info(layer, t + w)
            first_b.setdefault(key, (i, w))
            last_b[key] = (i, w)

    nc = bacc.Bacc("TRN2", target_bir_lowering=False, debug=False,
                   num_devices=NCORES)
    x_d = nc.dram_tensor("x", [N_SRC0, D], BF16, kind="ExternalInput")
    idxs_d = nc.dram_tensor("idxs", [128, nidxcol], I16, kind="ExternalInput")
    drv_d = nc.dram_tensor("drv", [128, max(nv, 1)], F32, kind="ExternalInput")
    ewv_d = nc.dram_tensor("ewv", [128, max(nv, 1)], F32, kind="ExternalInput")
    drna_d = nc.dram_tensor("drna", [128, max(na, 1)], F32,
                            kind="ExternalInput")
    ewa_d = nc.dram_tensor("ewa", [128, max(na, 1)], F32,
                           kind="ExternalInput")
    ewna_d = nc.dram_tensor("ewna", [128, max(na, 1)], F32,
                            kind="ExternalInput")
    h0_d = nc.dram_tensor("h0", [SLICE0, D], BF16)
    out_d = nc.dram_tensor("part", [T1 * 128, D], F32, kind="ExternalOutput")

    with tile.TileContext(nc) as tc:
        with (
            tc.tile_pool(name="const", bufs=1) as cpool,
            tc.tile_pool(name="mpool", bufs=6) as mpool,
            tc.tile_pool(name="spool", bufs=12) as spool,
            tc.tile_pool(name="sqpool", bufs=6) as sqpool,
            tc.tile_pool(name="stage", bufs=2) as stpool,
            tc.tile_pool(name="psum", bufs=1, space="PSUM") as ppool,
        ):
            iota32 = cpool.tile([128, 256], I32)
            iotabf = cpool.tile([128, 256], BF16)
            nc.gpsimd.iota(iota32[:], pattern=[[1, 256]], base=0,
                           channel_multiplier=0)
            nc.vector.tensor_copy(iotabf[:], iota32[:])

            idxs = cpool.tile([128, nidxcol], I16)
            drv = cpool.tile([128, max(nv, 1)], F32)
            ewv = cpool.tile([128, max(nv, 1)], F32)
            drna = cpool.tile([128, max(na, 1)], F32)
            ewa = cpool.tile([128, max(na, 1)], F32)
            ewna = cpool.tile([128, max(na, 1)], F32)
            # interleave loads so the first chunk of every array lands early;
            # the very first chunk of each array is small to unblock compute
            NLOAD = 6
            chunks = []
            for t_, d_ in ((idxs, idxs_d), (drv, drv_d), (ewv, ewv_d),
                           (drna, drna_d), (ewa, ewa_d), (ewna, ewna_d)):
                n = t_.shape[1]
                first = max(1, n // 24)
                step = -(-(n - first) // (NLOAD - 1))
                cl = [(t_, d_, 0, min(first, n))]
                for i in range(NLOAD - 1):
                    a = first + i * step
                    b = min(first + (i + 1) * step, n)
                    if a < b:
                        cl.append((t_, d_, a, b))
                chunks.append(cl)
            for i in range(NLOAD):
                for cl in chunks:
                    if i < len(cl):
                        t_, d_, a, b = cl[i]
                        nc.sync.dma_start(t_[:, a:b], d_[:, a:b])

            h0acc = cpool.tile([128, T0 * 128], BF16)

            banks = {}

            def get_bank(key):
                if key not in banks:
                    banks[key] = [ppool.tile([128, 512], F32,
                                             name=f"bk{key[2]}"),
                                  False]
                return banks[key][0]

            # walk builds in order; manage spans/gathers lazily
            mtiles = {}
            vi = 0
            ai = 0

            def ensure_span(si):
                if si in mtiles:
                    return mtiles[si]
                layer, table, a, b = spans[si]
                L = b - a
                mt = mpool.tile([128, CALL_G, 128], BF16, name="mt")
                tbl = x_d[table * CHUNK:(table + 1) * CHUNK, :] if layer == 0 \
                    else h0_d[:]
                ic = span_icol[si]
                nc.gpsimd.dma_gather(
                    mt[:, :L, :], tbl, idxs[:, ic:ic + L * 8],
                    num_idxs=L * 128, num_idxs_reg=L * 128, elem_size=128)
                mtiles.clear()
                mtiles[si] = mt
                return mt

            copy_after = {}     # build index -> list of copy ops
            # L0: whole-bank copies at end of each pass
            lastb_pass = {}
            for i, (layer, g, t, eng, width) in enumerate(builds):
                if layer == 0:
                    p = 0 if t < PASS0 else 1
                    lastb_pass[p] = i
            for p in (0, 1):
                ntile = PASS0 if p == 0 else T0 - PASS0
                nbank = -(-ntile // 4)
                ops = []
                for b in range(nbank):
                    ncols = min(4, ntile - b * 4) * 128
                    ops.append(('L0', p, b, ncols))
                ops.append(('H0', p))
                copy_after.setdefault(lastb_pass[p], []).extend(ops)
            # L1: full-bank copies, at the bank's last build
            lastb_t1 = {}
            for i, (layer, g, t, eng, width) in enumerate(builds):
                if layer == 1:
                    for w in range(width):
                        lastb_t1[t + w] = i
            for jb in range(T1 // 4):
                i = max(lastb_t1[4 * jb + k] for k in range(4))
                copy_after.setdefault(i, []).append(('L1', jb))

            stage_tiles = {}

            def do_copies(items):
                for op in items:
                    if op[0] == 'L0':
                        _, p, b, ncols = op
                        base = (0 if p == 0 else PASS0) * 128
                        bk = banks[(0, p, b)][0]
                        a = base + b * 512
                        nc.scalar.activation(
                            h0acc[:, a: a + ncols],
                            bk[:, :ncols], AF.Copy, bias=0.0, scale=1.0)
                        # store this bank's h0 rows immediately so the L1
                        # gather table completes as soon as possible
                        dram = h0_d[a:a + ncols, :].rearrange(
                            "(t p) d -> p t d", p=128)
                        nc.sync.dma_start(
                            dram, h0acc[:, a:a + ncols].rearrange(
                                "p (t d) -> p t d", d=128))
                    elif op[0] == 'H0':
                        pass
                    else:
                        _, jb = op
                        t0 = 4 * jb
                        key, q0 = bank_info(1, t0)
                        bk = banks[key][0]
                        jblk = t0 // STAGE_T
                        lt = t0 % STAGE_T
                        if jblk not in stage_tiles:
                            stage_tiles[jblk] = stpool.tile(
                                [128, STAGE_T * 128], F32, name="stg")
                        nc.scalar.activation(
                            stage_tiles[jblk][:, lt * 128:(lt + 4) * 128],
                            bk[:, :512],
                            AF.Copy, bias=0.0, scale=1.0)
                        last_blk = jblk == T1 // STAGE_T - 1
                        parts = ((12, 0, 16), (20, 16, 24),
                                 (STAGE_T - 4, 24, STAGE_T)) \
                            if last_blk else ((STAGE_T - 4, 0, STAGE_T),)
                        for (trig, c0, c1) in parts:
                            if lt != trig:
                                continue
                            rows = STAGE_T * 128
                            dram = out_d[jblk * rows + c0 * 128:
                                         jblk * rows + c1 * 128, :] \
                                .rearrange("(t p) d -> p t d", p=128)
                            nc.sync.dma_start(
                                dram,
                                stage_tiles[jblk][:, c0 * 128:c1 * 128]
                                .rearrange("p (t d) -> p t d", d=128))
                            if c1 == STAGE_T:
                                del stage_tiles[jblk]

            for i, (layer, g, t, eng, width) in enumerate(builds):
                si, col = g_span[g]
                mt = ensure_span(si)
                nw = width * 128
                if eng == 'V':
                    S = spool.tile([128, 256], BF16, name="Sv")
                    nc.vector.tensor_scalar(
                        S[:, :nw], iotabf[:, :nw], drv[:, vi:vi + 1],
                        ewv[:, vi:vi + 1],
                        mybir.AluOpType.is_equal, mybir.AluOpType.mult)
                    vi += 1
                else:
                    sq = sqpool.tile([128, 256], BF16, name="sq")
                    nc.scalar.activation(sq[:, :nw], iotabf[:, :nw], AF.Square,
                                         bias=drna[:, ai:ai + 1], scale=1.0)
                    S = spool.tile([128, 256], BF16, name="Sa")
                    nc.scalar.activation(S[:, :nw], sq[:, :nw], AF.Relu,
                                         bias=ewa[:, ai:ai + 1],
                                         scale=ewna[:, ai:ai + 1])
                    ai += 1
                for w in range(width):
                    key, q = bank_info(layer, t + w)
                    bk = get_bank(key)
                    nc.tensor.matmul(bk[:, q * 128:(q + 1) * 128],
                                     S[:, w * 128:(w + 1) * 128],
                                     mt[:, col, :],
                                     start=(first_b[key] == (i, w)),
                                     stop=(last_b[key] == (i, w)))
                if i in copy_after:
                    do_copies(copy_after[i])

    nc.compile()
    return nc


def kernel(x, src0, dst0, ew0, src1, dst1, ew1, n_dst0, n_dst1):
    global _last_results, _last_nc
    t_start = time.time()
    x = np.asarray(x, dtype=np.float32)
    src0 = np.asarray(src0).astype(np.int64)
    dst0 = np.asarray(dst0).astype(np.int64)
    ew0 = np.asarray(ew0, dtype=np.float32)
    src1 = np.asarray(src1).astype(np.int64)
    dst1 = np.asarray(dst1).astype(np.int64)
    ew1 = np.asarray(ew1, dtype=np.float32)

    x_bf = x.astype(ml_dtypes.bfloat16)

    pk, spans, in_maps = _pack(x_bf, src0, dst0, ew0, src1, dst1, ew1)
    nv = len(pk.drv)
    na = len(pk.drna)
    nidxcol = in_maps[0]["idxs"].shape[1]
    t_pack = time.time()

    nc = _build_program(pk, spans, nv, na, nidxcol)
    _last_nc = nc
    t_build = time.time()

    trace = bool(int(os.environ.get("KBENCH_TRACE", "0")))
    try:
        res = run_bass_kernel_spmd(nc, in_maps, list(range(NCORES)),
                                   trace=trace)
    except ModuleNotFoundError:
        res = run_bass_kernel_spmd(nc, in_maps, list(range(NCORES)),
                                   trace=False)
    _last_results = res
    t_run = time.time()
    print(f"[kernel] pack {t_pack - t_start:.1f}s build+compile "
          f"{t_build - t_pack:.1f}s run {t_run - t_build:.1f}s "
          f"groups={pk.g_total} builds={len(pk.builds)} nv={nv} na={na}",
          file=sys.stderr)

    out = np.zeros((T1 * 128, D), np.float32)
    for c in range(NCORES):
        out += res.results[c]["part"]
    return out[: int(n_dst1)]
